# revision 2
# baseline (speedup 1.0000x reference)
"""Bass/Trainium2 SPMD kernel for nn_Block3D (8 NeuronCores) — v2.

z-shard (3 planes/core) with a 2-deep halo (7 input planes) so the LN2
halo exchange is computed locally instead of AllGathered. Collectives are
three small AllGathers (vc partial sums, kernel_net rows, GN stats) with
local reduction. The whole MLP (Wi / depthwise conv / Wo) runs in fp8
DoubleRow (tap-pair matmuls, 4x MAC rate); the CAFM dynamic conv stays
bf16 diag-matmul on PE with DVE-built diagonal stationaries.
"""

from contextlib import ExitStack

import numpy as np
import ml_dtypes

import concourse.bass as bass
import concourse.bacc as bacc
import concourse.tile as tile
from concourse import mybir
from concourse.bass_utils import run_bass_kernel_spmd

BF = ml_dtypes.bfloat16
E4 = ml_dtypes.float8_e4m3fn
F32 = mybir.dt.float32
BF16 = mybir.dt.bfloat16
FP8 = mybir.dt.float8e4

C = 768
G = 12
GD = 64
S = 24
HID = 4 * C          # 3072
HT = HID // 128      # 24
HH = HT // 2         # 12
CT = C // 128        # 6
KK = 27
V = S * S * S
EPS = 1e-5
NCORES = 8
ZP = S // NCORES     # 3 owned planes
PL = S * S           # 576
VC = ZP * PL         # 1728 owned voxels
Z7 = ZP + 4          # 7 input planes
Z5 = ZP + 2          # 5 planes for xb/xln
W5 = Z5 * PL         # 2880
PPL = 26 * 26        # 676 padded plane
HDR = 32             # headroom around padded slabs (keeps offsets even)
MVN = HDR + Z7 * PPL + HDR
HPN = HDR + Z5 * PPL + HDR
KFLAT = C * KK       # 20736
W2R = KFLAT // NCORES  # 2592
NB = 288
WSC = 16.0           # fp8 weight scale
GSC = 256.0          # fp8 gate scale

TAPS = [(dz, dy, dx) for dz in (-1, 0, 1) for dy in (-1, 0, 1) for dx in (-1, 0, 1)]


def _tidx(dz, dy, dx):
    return (dz + 1) * 9 + (dy + 1) * 3 + (dx + 1)


# DoubleRow tap pairs: the pair stride (byte delta between the two moving
# windows) must be EVEN, so pair dx=-1 with dx=+1 (delta 2), and the dx=0
# column across dz/dy (deltas 676 / 26).
PAIRS = ([(_tidx(dz, dy, -1), _tidx(dz, dy, 1))
          for dz in (-1, 0, 1) for dy in (-1, 0, 1)]
         + [(_tidx(-1, dy, 0), _tidx(0, dy, 0)) for dy in (-1, 0, 1)]
         + [(_tidx(1, -1, 0), _tidx(1, 0, 0)), (_tidx(1, 1, 0), None)])
NPR = len(PAIRS)     # 14

_CACHE = {}

Copy = mybir.ActivationFunctionType.Copy
Iden = mybir.ActivationFunctionType.Identity
Gelu = mybir.ActivationFunctionType.Gelu
Sigmoid = mybir.ActivationFunctionType.Sigmoid
Square = mybir.ActivationFunctionType.Square
Sqrt = mybir.ActivationFunctionType.Sqrt
Relu = mybir.ActivationFunctionType.Relu
ADD = mybir.AluOpType.add
SUB = mybir.AluOpType.subtract
MULT = mybir.AluOpType.mult
DR = mybir.MatmulPerfMode.DoubleRow

# smalls columns
(S_Y, S_BA, S_BB, S_BC, S_MODB, S_OPB, S_N2W, S_N2B, S_N3W, S_N3B,
 S_GNG, S_GNB, S_N2WL, S_N2BL, S_N2WH, S_N2BH) = range(16)


def _toff(dz, dy, dx):
    return dz * PPL + dy * 26 + dx


def build_program():
    nc = bacc.Bacc("TRN2", target_bir_lowering=False)

    def dram_in(name, shape, dtype=F32):
        return nc.declare_dram_parameter(name, list(shape), dtype, isOutput=False)

    x_halo = dram_in("x_halo", [C, Z7 * PL], BF16)
    smalls = dram_in("smalls", [C, 16])
    knb1 = dram_in("knb1", [HID])
    knb2r = dram_in("knb2r", [KFLAT])
    gind6 = dram_in("gind6", [CT, 128, G])
    gexpT = dram_in("gexpT", [G, C])
    ident = dram_in("ident", [128, 128], BF16)
    onesc = dram_in("onesc", [128, 1], BF16)
    wAT = dram_in("wAT", [C, C], BF16)
    wBT = dram_in("wBT", [C, C], BF16)
    wCT = dram_in("wCT", [C, C], BF16)
    modWT = dram_in("modWT", [2 * C, C], BF16)
    w1T = dram_in("w1T", [2 * C, HID], BF16)
    w2m = dram_in("w2m", [HT, 128, W2R], BF16)
    opT = dram_in("opT", [C, C], BF16)
    wi8 = dram_in("wi8", [HT, 128, 3 * 256], FP8)
    d8 = dram_in("d8", [HT, 128, NPR * 256], FP8)
    wo8 = dram_in("wo8", [CT, 128, 6 * 256], FP8)
    out = nc.declare_dram_parameter("out", [C, VC], F32, isOutput=True)
    import os as _os
    DBG = _os.environ.get("BLK3D_DBG") == "1"
    if DBG:
        dbg_mod = nc.declare_dram_parameter("dbg_mod", [C, 1], F32, isOutput=True)
        dbg_kern = nc.declare_dram_parameter("dbg_kern", [C, KK], F32, isOutput=True)
        dbg_dyn = nc.declare_dram_parameter("dbg_dyn", [C, W5], F32, isOutput=True)
        dbg_xb = nc.declare_dram_parameter("dbg_xb", [C, W5], F32, isOutput=True)
        dbg_xln = nc.declare_dram_parameter("dbg_xln", [3 * 128, 2 * W5], F32, isOutput=True)
        dbg_gate = nc.declare_dram_parameter("dbg_gate", [6 * 128, 2 * VC], F32, isOutput=True)
        dbg_y = nc.declare_dram_parameter("dbg_y", [C, VC], F32, isOutput=True)
        dbg_comb = nc.declare_dram_parameter("dbg_comb", [C, 2], F32, isOutput=True)
        dbg_kp1 = nc.declare_dram_parameter("dbg_kp1", [HID, 1], F32, isOutput=True)
        dbg_krow = nc.declare_dram_parameter("dbg_krow", [1, W2R], F32, isOutput=True)

    with tile.TileContext(nc) as tc, ExitStack() as ctx:
        dram = ctx.enter_context(tc.tile_pool(name="dram", bufs=1, space="DRAM"))
        persist = ctx.enter_context(tc.tile_pool(name="persist", bufs=1))
        gpool = ctx.enter_context(tc.tile_pool(name="gemv", bufs=2))
        psA = ctx.enter_context(tc.tile_pool(name="psA", bufs=2, space="PSUM"))

        # ---------------- persistent small tiles ----------------
        sm = [persist.tile([128, 16], F32, name=f"sm{i}", tag=f"sm{i}")
              for i in range(CT)]
        for i in range(CT):
            nc.sync.dma_start(sm[i][:], smalls[128 * i:128 * (i + 1), :])
        id_t = persist.tile([128, 128], BF16, name="identt", tag="identt")
        nc.sync.dma_start(id_t[:], ident[:, :])
        ones_t = persist.tile([128, 1], BF16, name="onest", tag="onest")
        nc.sync.dma_start(ones_t[:], onesc[:, :])
        eps_t = persist.tile([128, 1], F32, name="epst", tag="epst")
        nc.vector.memset(eps_t[:], EPS)
        junk = persist.tile([128, VC], BF16, name="junk", tag="junk")
        kern = [persist.tile([128, KK], F32, name=f"kern{i}", tag=f"kern{i}")
                for i in range(CT)]
        cb_cols = [persist.tile([128, 1], F32, name=f"cbc{m}", tag=f"cbc{m}")
                   for m in range(CT)]

        xbp = ctx.enter_context(tc.tile_pool(name="xbp", bufs=1))
        xb = [xbp.tile([128, W5], BF16, name=f"xb{i}", tag=f"xb{i}")
              for i in range(CT)]

        def ln_rows(pool, tiles, width, tag):
            """per-voxel mean/rstd over C -> bf16 bcast tiles [128, width]"""
            with (tc.tile_pool(name=f"{tag}ps", bufs=2, space="PSUM") as lps,
                  tc.tile_pool(name=f"{tag}sq", bufs=2) as sqp,
                  tc.tile_pool(name=f"{tag}rw", bufs=1) as rwp):
                nch = (width + 511) // 512
                row = rwp.tile([1, 2 * width], F32, name="row", tag="row")
                for cidx in range(nch):
                    o0 = 512 * cidx
                    n = min(512, width - o0)
                    ps1 = lps.tile([1, 512], F32, name="s1", tag="s1")
                    for k in range(CT):
                        nc.tensor.matmul(ps1[:, 0:n], ones_t[:],
                                         tiles[k][:, o0:o0 + n],
                                         start=(k == 0), stop=(k == CT - 1))
                    nc.scalar.activation(row[:, o0:o0 + n], ps1[:, 0:n], Copy,
                                         scale=1.0 / C)
                for cidx in range(nch):
                    o0 = 512 * cidx
                    n = min(512, width - o0)
                    ps2 = lps.tile([1, 512], F32, name="s2", tag="s2")
                    for k in range(CT):
                        sq = sqp.tile([128, 512], BF16, name="sq", tag="sq")
                        nc.vector.tensor_mul(sq[:, 0:n], tiles[k][:, o0:o0 + n],
                                             tiles[k][:, o0:o0 + n])
                        nc.tensor.matmul(ps2[:, 0:n], ones_t[:], sq[:, 0:n],
                                         start=(k == 0), stop=(k == CT - 1))
                    nc.scalar.activation(row[:, width + o0:width + o0 + n],
                                         ps2[:, 0:n], Copy, scale=1.0 / C)
                # spread each half to [96, w] for cheap elementwise math
                w96 = width // 96
                rs = rwp.tile([96, 2 * w96], F32, name="rs", tag="rs")
                nc.gpsimd.dma_start(rs[:, 0:w96], row[:, 0:width])
                nc.gpsimd.dma_start(rs[:, w96:2 * w96], row[:, width:2 * width])
                m2 = rwp.tile([96, w96], F32, name="m2", tag="m2")
                nc.scalar.square(m2[:], rs[:, 0:w96])
                vr = rwp.tile([96, w96], F32, name="vr", tag="vr")
                nc.vector.tensor_sub(vr[:], rs[:, w96:2 * w96], m2[:])
                nc.scalar.activation(vr[:], vr[:], Sqrt, bias=eps_t[0:96, 0:1])
                nc.vector.reciprocal(vr[:], vr[:])
                mrow = rwp.tile([1, 2 * width], BF16, name="mrow", tag="mrow")
                nc.gpsimd.dma_start(mrow[:, 0:width], rs[:, 0:w96])
                nc.gpsimd.dma_start(mrow[:, width:2 * width], vr[:])
                muB = pool.tile([128, width], BF16, name=f"{tag}mu",
                                tag=f"{tag}mu")
                rsB = pool.tile([128, width], BF16, name=f"{tag}rs",
                                tag=f"{tag}rs")
                nc.gpsimd.partition_broadcast(muB[:], mrow[0:1, 0:width])
                nc.gpsimd.partition_broadcast(rsB[:], mrow[0:1, width:2 * width])
            return muB, rsB

        # =================== phases A-C: scoped (xs/mv/dyn die after) ======
        with (tc.tile_pool(name="xsp", bufs=1) as xsp,
              tc.tile_pool(name="dynp", bufs=1) as dynp,
              tc.tile_pool(name="opTp", bufs=1) as opTp):
            xs = [xsp.tile([128, Z7 * PL], BF16, name=f"xs{i}", tag=f"xs{i}")
                  for i in range(CT)]
            for i in range(CT):
                nc.sync.dma_start(xs[i][:], x_halo[128 * i:128 * (i + 1), :])
            dyn = [dynp.tile([128, W5], BF16, name=f"dyn{i}", tag=f"dyn{i}")
                   for i in range(CT)]
            opT_t = [opTp.tile([128, C], BF16, name=f"opT{i}", tag=f"opT{i}")
                     for i in range(CT)]
            for i in range(CT):
                nc.sync.dma_start(opT_t[i][:], opT[128 * i:128 * (i + 1), :])

            # ---- vc partials + AG1 ----
            vcs = persist.tile([128, CT], F32, name="vcs", tag="vcs")
            for i in range(CT):
                nc.scalar.activation(junk[:], xs[i][:, 2 * PL:2 * PL + VC],
                                     Copy, accum_out=vcs[:, i:i + 1])
            ag1i = dram.tile([C], F32, name="ag1i", tag="ag1i")
            ag1o = dram.tile([NCORES, C], F32, name="ag1o", tag="ag1o",
                             addr_space="Shared")
            nc.gpsimd.dma_start(
                bass.AP(tensor=ag1i[:].tensor, offset=ag1i[:].offset,
                        ap=[[1, 128], [128, CT]]), vcs[:])
            nc.gpsimd.collective_compute(
                "AllGather", mybir.AluOpType.bypass,
                replica_groups=[list(range(NCORES))], ins=[ag1i[:]],
                outs=[ag1o[:]])

            # ---- text chain / mod / kp1 (row-form gemv) ----
            _psRs = ExitStack()
            psR = _psRs.enter_context(
                tc.tile_pool(name="psR", bufs=1, space="PSUM"))
            pcs = [psR.tile([1, 512], F32, name=f"rp{j}", tag=f"rowps{j}")
                   for j in range(6)]
            with tc.tile_pool(name="wstream", bufs=2) as wpool:

                def gemv_chain(wdram, in_cols, nk, nm, act, bias_col, tag,
                               odt=BF16, bias_t=None):
                    # out = W @ in  via moving-weights row matmuls; the row
                    # [1, 128*nm] is transposed to columns through DRAM.
                    width = 128 * nm
                    npc = (width + 511) // 512
                    for k in range(nk):
                        wt = wpool.tile([128, width], BF16, name=f"{tag}w",
                                        tag=f"{tag}w")
                        nc.sync.dma_start(wt[:],
                                          wdram[128 * k:128 * (k + 1), :])
                        for pc in range(npc):
                            o0 = 512 * pc
                            n = min(512, width - o0)
                            nc.tensor.matmul(pcs[pc][:, 0:n], in_cols[k][:],
                                             wt[:, o0:o0 + n],
                                             start=(k == 0),
                                             stop=(k == nk - 1))
                    row = gpool.tile([1, width], BF16, name=f"{tag}row",
                                     tag=f"{tag}row")
                    for pc in range(npc):
                        o0 = 512 * pc
                        n = min(512, width - o0)
                        nc.scalar.activation(row[:, o0:o0 + n],
                                             pcs[pc][:, 0:n], Copy)
                    drow = dram.tile([width], BF16, name=f"{tag}dr",
                                     tag=f"{tag}dr")
                    nc.gpsimd.dma_start(drow[:], row[:])
                    ct = gpool.tile([128, nm], BF16, name=f"{tag}ct",
                                    tag=f"{tag}ct")
                    nc.gpsimd.dma_start(
                        ct[:], bass.AP(tensor=drow[:].tensor,
                                       offset=drow[:].offset,
                                       ap=[[1, 128], [128, nm]]))
                    outs = []
                    for m in range(nm):
                        if bias_t is not None:
                            bias = bias_t[:, m:m + 1]
                        else:
                            bias = sm[m][:, bias_col:bias_col + 1]
                        o = gpool.tile([128, 1], odt, name=f"{tag}o{m}",
                                       tag=f"{tag}o{m}")
                        nc.scalar.activation(o[:], ct[:, m:m + 1], act,
                                             bias=bias)
                        outs.append(o)
                    return outs

                y_cols = []
                for i in range(CT):
                    t = gpool.tile([128, 1], BF16, name=f"yc{i}", tag=f"yc{i}")
                    nc.scalar.activation(t[:], sm[i][:, S_Y:S_Y + 1], Copy)
                    y_cols.append(t)
                hA = gemv_chain(wAT, y_cols, CT, CT, Relu, S_BA, "wa")
                hB = gemv_chain(wBT, hA, CT, CT, Iden, S_BB, "wb")
                attn = gemv_chain(wCT, hB, CT, CT, Iden, S_BC, "wc")

                vc8 = persist.tile([128, CT * NCORES], F32, name="vc8",
                                   tag="vc8")
                nc.gpsimd.dma_start(
                    bass.AP(tensor=vc8[:].tensor, offset=vc8[:].offset,
                            ap=[vc8[:].ap[0], [CT, NCORES], [1, CT]]),
                    bass.AP(tensor=ag1o[:].tensor, offset=ag1o[:].offset,
                            ap=[[1, 128], [C, NCORES], [128, CT]]))
                v3 = vc8.rearrange("p (i k) -> p i k", i=NCORES, k=CT)
                for step in (4, 2, 1):
                    nc.vector.tensor_add(v3[:, 0:step, :], v3[:, 0:step, :],
                                         v3[:, step:2 * step, :])
                comb = []
                for i in range(CT):
                    cb = gpool.tile([128, 1], BF16, name=f"cmb{i}",
                                    tag=f"cmb{i}")
                    nc.scalar.activation(cb[:], vc8[:, i:i + 1], Copy,
                                         scale=1.0 / V)
                    comb.append(cb)
                comb += attn

                mod = gemv_chain(modWT, comb, 2 * CT, CT, Sigmoid, S_MODB,
                                 "md", odt=F32)

                knb1_t = persist.tile([128, HT], F32, name="knb1t",
                                      tag="knb1t")
                nc.gpsimd.dma_start(
                    knb1_t[:],
                    bass.AP(tensor=knb1, offset=0, ap=[[1, 128], [128, HT]]))
                kp1 = gemv_chain(w1T, comb, 2 * CT, HT, Relu, None, "k1",
                                 bias_t=knb1_t)

            if DBG:
                with tc.tile_pool(name="dbg0p", bufs=2) as dbg0p:
                    for i in range(CT):
                        t = dbg0p.tile([128, 2], F32, name="dbgc", tag="dbgc")
                        nc.scalar.activation(t[:, 0:1], comb[i][:], Copy)
                        nc.scalar.activation(t[:, 1:2], comb[CT + i][:], Copy)
                        nc.gpsimd.dma_start(dbg_comb[128 * i:128 * (i + 1), :],
                                            t[:])
                    for m in range(HT):
                        t = dbg0p.tile([128, 1], F32, name="dbgk1", tag="dbgk1")
                        nc.scalar.activation(t[:], kp1[m][:], Copy)
                        nc.gpsimd.dma_start(
                            dbg_kp1[128 * m:128 * (m + 1), :], t[:])

            # ---- W2 own rows + AG2 -> kernels ----
            ag2i = dram.tile([W2R], F32, name="ag2i", tag="ag2i")
            ag2o = dram.tile([NCORES, W2R], F32, name="ag2o", tag="ag2o",
                             addr_space="Shared")
            PIECES = [(0, 512), (512, 512), (1024, 512), (1536, 512),
                      (2048, 512), (2560, 32)]
            with (tc.tile_pool(name="w2s", bufs=3) as w2s,
                  tc.tile_pool(name="krowp", bufs=1) as krowp):
                pcs = [psR.tile([1, 512], F32, name=f"w2p{j}",
                                tag=f"rowps{j}") for j in range(6)]
                for k in range(HT):
                    wt = w2s.tile([128, W2R], BF16, name="w2w", tag="w2w")
                    nc.sync.dma_start(wt[:], w2m[k, :, :])
                    for j, (o0, n) in enumerate(PIECES):
                        nc.tensor.matmul(pcs[j][:, 0:n], kp1[k][:],
                                         wt[:, o0:o0 + n], start=(k == 0),
                                         stop=(k == HT - 1))
                krow = krowp.tile([1, W2R], F32, name="krow", tag="krow")
                for j, (o0, n) in enumerate(PIECES):
                    nc.scalar.activation(krow[:, o0:o0 + n], pcs[j][:, 0:n],
                                         Copy)
                nc.gpsimd.dma_start(ag2i[:], krow[:])
                if DBG:
                    nc.gpsimd.dma_start(dbg_krow[:, :], krow[:])
            nc.gpsimd.collective_compute(
                "AllGather", mybir.AluOpType.bypass,
                replica_groups=[list(range(NCORES))], ins=[ag2i[:]],
                outs=[ag2o[:]])
            for i in range(CT):
                nc.gpsimd.dma_start(
                    kern[i][:],
                    bass.AP(tensor=ag2o[:].tensor,
                            offset=ag2o[:].offset + 128 * i * KK,
                            ap=[[KK, 128], [1, KK]]))
                kb = gpool.tile([128, KK], F32, name="kernb", tag="kernb")
                nc.gpsimd.dma_start(
                    kb[:], bass.AP(tensor=knb2r, offset=128 * i * KK,
                                   ap=[[KK, 128], [1, KK]]))
                nc.vector.tensor_add(kern[i][:], kern[i][:], kb[:])
            _psRs.close()

            # ---- mv staging + dyn conv + GN stats ----
            ag3i = dram.tile([G, 2], F32, name="ag3i", tag="ag3i")
            ag3o = dram.tile([NCORES, G, 2], F32, name="ag3o", tag="ag3o",
                             addr_space="Shared")
            with (tc.tile_pool(name="mvp", bufs=2) as mvp,
                  tc.tile_pool(name="diagp", bufs=2) as diagp,
                  tc.tile_pool(name="cvps", bufs=4, space="PSUM") as cvps,
                  tc.tile_pool(name="gnps", bufs=1, space="PSUM") as gnps,
                  tc.tile_pool(name="gnst", bufs=2) as gnst):
                gps = gnps.tile([G, 2], F32, name="gps", tag="gps")
                for i in range(CT):
                    mv = mvp.tile([128, MVN], BF16, name="mv", tag="mv")
                    if i < 2:
                        nc.gpsimd.memset(mv[:], 0.0)
                    for z in range(Z7):
                        for yh in range(2):
                            dst = bass.AP(
                                tensor=mv[:].tensor,
                                offset=(mv[:].offset + HDR + z * PPL
                                        + yh * 312),
                                ap=[mv[:].ap[0], [26, 12], [1, 24]])
                            nc.vector.tensor_scalar_mul(
                                dst,
                                xs[i][:, z * PL + yh * NB:
                                      z * PL + yh * NB + NB],
                                mod[i][:, 0:1])
                    dg = [diagp.tile([128, 128], BF16, name=f"dg{t}",
                                     tag=f"dg{t}") for t in range(KK)]
                    for t in range(KK):
                        nc.vector.tensor_scalar_mul(dg[t][:], id_t[:],
                                                    kern[i][:, t:t + 1])
                    for o in range(Z5):
                        for bh in range(2):
                            ps = cvps.tile([128, NB], F32, name="cv",
                                           tag="cv")
                            base = (mv[:].offset + HDR + o * PPL + bh * 312)
                            for t, (dz, dy, dx) in enumerate(TAPS):
                                sv = bass.AP(
                                    tensor=mv[:].tensor,
                                    offset=base + _toff(1 + dz, dy, dx),
                                    ap=[mv[:].ap[0], [26, 12], [1, 24]])
                                nc.tensor.matmul(ps[:], dg[t][:], sv,
                                                 start=(t == 0),
                                                 stop=(t == KK - 1))
                            nc.scalar.activation(
                                dyn[i][:, o * PL + bh * NB:
                                       o * PL + bh * NB + NB],
                                ps[:], Copy)
                    st = gnst.tile([128, 2], F32, name="gnstat", tag="gnstat")
                    nc.scalar.activation(junk[:], dyn[i][:, PL:PL + VC], Copy,
                                         accum_out=st[:, 0:1])
                    nc.scalar.activation(junk[:], dyn[i][:, PL:PL + VC],
                                         Square, accum_out=st[:, 1:2])
                    gi = gnst.tile([128, G], F32, name="gind", tag="gind")
                    nc.gpsimd.dma_start(gi[:], gind6[i, :, :])
                    nc.tensor.matmul(gps[:], gi[:], st[:], start=(i == 0),
                                     stop=(i == CT - 1))
                gsb = persist.tile([G, 2], F32, name="gsb", tag="gsb")
                nc.scalar.activation(gsb[:], gps[:], Copy)
            nc.gpsimd.dma_start(ag3i[:], gsb[:])
            nc.gpsimd.collective_compute(
                "AllGather", mybir.AluOpType.bypass,
                replica_groups=[list(range(NCORES))], ins=[ag3i[:]],
                outs=[ag3o[:]])

            gst8 = persist.tile([G, 2 * NCORES], F32, name="gst8", tag="gst8")
            nc.gpsimd.dma_start(
                bass.AP(tensor=gst8[:].tensor, offset=gst8[:].offset,
                        ap=[gst8[:].ap[0], [2, NCORES], [1, 2]]),
                bass.AP(tensor=ag3o[:].tensor, offset=ag3o[:].offset,
                        ap=[[2, G], [2 * G, NCORES], [1, 2]]))
            g3 = gst8.rearrange("p (i t) -> p i t", i=NCORES, t=2)
            for step in (4, 2, 1):
                nc.vector.tensor_add(g3[:, 0:step, :], g3[:, 0:step, :],
                                     g3[:, step:2 * step, :])
            NGRP = float(GD * V)
            gmr = persist.tile([G, 2], F32, name="gmr", tag="gmr")
            nc.scalar.activation(gmr[:, 0:1], gst8[:, 0:1], Copy,
                                 scale=1.0 / NGRP)
            musq = persist.tile([G, 1], F32, name="musq", tag="musq")
            nc.scalar.square(musq[:], gmr[:, 0:1])
            var = persist.tile([G, 1], F32, name="gvar", tag="gvar")
            nc.vector.tensor_scalar(var[:], gst8[:, 1:2], 1.0 / NGRP, None,
                                    op0=MULT)
            nc.vector.tensor_sub(var[:], var[:], musq[:])
            nc.scalar.activation(var[:], var[:], Sqrt, bias=eps_t[0:G, 0:1])
            nc.vector.reciprocal(gmr[:, 1:2], var[:])

            with tc.tile_pool(name="gnf", bufs=2) as gnf:
                shifts = []
                gscs = []
                for i in range(CT):
                    ge = gnf.tile([G, 128], F32, name=f"gexp{i}",
                                  tag=f"gexp{i}")
                    nc.gpsimd.dma_start(ge[:], gexpT[:, 128 * i:128 * (i + 1)])
                    ps = psA.tile([128, 2], F32, name="gn2", tag="gvps")
                    nc.tensor.matmul(ps[:], ge[:], gmr[:], start=True,
                                     stop=True)
                    mu_c = gnf.tile([128, 2], F32, name=f"muc{i}",
                                    tag=f"muc{i}")
                    nc.scalar.activation(mu_c[:], ps[:], Copy)
                    a = persist.tile([128, 1], F32, name=f"gsc{i}",
                                     tag=f"gsc{i}")
                    nc.vector.tensor_mul(a[:], sm[i][:, S_GNG:S_GNG + 1],
                                         mu_c[:, 1:2])
                    b = gnf.tile([128, 1], BF16, name=f"gsh{i}", tag=f"gsh{i}")
                    t = gnf.tile([128, 1], F32, name="gtmp", tag="gtmp")
                    nc.vector.tensor_mul(t[:], mu_c[:, 0:1], a[:])
                    nc.vector.tensor_sub(t[:], sm[i][:, S_GNB:S_GNB + 1], t[:])
                    nc.scalar.activation(b[:], t[:], Copy)
                    shifts.append(b)
                    gscs.append(a)
                for m in range(CT):
                    ps = psA.tile([128, 1], F32, name="cbp", tag="gvps")
                    for k in range(CT):
                        nc.tensor.matmul(ps[:],
                                         opT_t[k][:, 128 * m:128 * (m + 1)],
                                         shifts[k][:], start=(k == 0),
                                         stop=(k == CT - 1))
                    nc.scalar.activation(cb_cols[m][:], ps[:], Iden,
                                         bias=sm[m][:, S_OPB:S_OPB + 1])
                for i in range(CT):
                    nc.vector.tensor_scalar_mul(opT_t[i][:], opT_t[i][:],
                                                gscs[i][:])

            # cafm matmul (5 planes) + xb = (psum + cb) * x
            CH5 = [(0, 512), (512, 512), (1024, 512), (1536, 512),
                   (2048, 512), (2560, 320)]
            with tc.tile_pool(name="opwps", bufs=2, space="PSUM") as opwps:
                for m in range(CT):
                    for o0, n in CH5:
                        ps = opwps.tile([128, 512], F32, name="opw", tag="opw")
                        for k in range(CT):
                            nc.tensor.matmul(
                                ps[:, 0:n],
                                opT_t[k][:, 128 * m:128 * (m + 1)],
                                dyn[k][:, o0:o0 + n], start=(k == 0),
                                stop=(k == CT - 1))
                        nc.vector.scalar_tensor_tensor(
                            xb[m][:, o0:o0 + n], ps[:, 0:n], cb_cols[m][:],
                            xs[m][:, PL + o0:PL + o0 + n], op0=ADD, op1=MULT)

            if DBG:
                with tc.tile_pool(name="dbgp", bufs=2) as dbgp:
                    for i in range(CT):
                        t = dbgp.tile([128, W5], F32, name="dbgt", tag="dbgt")
                        nc.scalar.activation(t[:], dyn[i][:], Copy)
                        nc.gpsimd.dma_start(dbg_dyn[128 * i:128 * (i + 1), :],
                                            t[:])
                        t2 = dbgp.tile([128, W5], F32, name="dbgt2",
                                       tag="dbgt2")
                        nc.scalar.activation(t2[:], xb[i][:], Copy)
                        nc.gpsimd.dma_start(dbg_xb[128 * i:128 * (i + 1), :],
                                            t2[:])
                        nc.gpsimd.dma_start(dbg_kern[128 * i:128 * (i + 1), :],
                                            kern[i][:])
                        t3 = dbgp.tile([128, 1], F32, name="dbgt3",
                                       tag="dbgt3")
                        nc.scalar.activation(t3[:], mod[i][:], Copy)
                        nc.gpsimd.dma_start(dbg_mod[128 * i:128 * (i + 1), :],
                                            t3[:])

        # =================== LN2 -> xln8 (fp8, channel-paired) =============
        xln8p = ctx.enter_context(tc.tile_pool(name="xln8p", bufs=1))
        xln8 = [xln8p.tile([128, 2 * W5], FP8, name=f"xl{q}", tag=f"xl{q}")
                for q in range(3)]
        with tc.tile_pool(name="lnbp", bufs=1) as lnbp:
            muB, rsB = ln_rows(lnbp, xb, W5, "ln2")
            with tc.tile_pool(name="lnt", bufs=2) as lnt:
                for i in range(CT):
                    t1 = lnt.tile([128, W5], BF16, name="lnt1", tag="lnt1")
                    nc.vector.tensor_sub(t1[:], xb[i][:], muB[:])
                    nc.vector.tensor_mul(t1[:], t1[:], rsB[:])
                    q, j = i // 2, i % 2
                    dst = xln8[q][:, j * W5:(j + 1) * W5]
                    nc.scalar.activation(dst[:, 0:PL], t1[:, 0:PL], Iden,
                                         bias=sm[i][:, S_N2BL:S_N2BL + 1],
                                         scale=sm[i][:, S_N2WL:S_N2WL + 1])
                    nc.scalar.activation(dst[:, PL:4 * PL], t1[:, PL:4 * PL],
                                         Iden,
                                         bias=sm[i][:, S_N2B:S_N2B + 1],
                                         scale=sm[i][:, S_N2W:S_N2W + 1])
                    nc.scalar.activation(dst[:, 4 * PL:5 * PL],
                                         t1[:, 4 * PL:5 * PL], Iden,
                                         bias=sm[i][:, S_N2BH:S_N2BH + 1],
                                         scale=sm[i][:, S_N2WH:S_N2WH + 1])

        # =================== MLP: fp8 DoubleRow ===========================
        gate8p = ctx.enter_context(tc.tile_pool(name="gate8p", bufs=1))
        gate8 = [gate8p.tile([128, 2 * VC], FP8, name=f"g8{q}", tag=f"g8{q}")
                 for q in range(6)]
        ytp = ctx.enter_context(tc.tile_pool(name="ytp", bufs=1))
        y_t = [ytp.tile([128, VC], BF16, name=f"y{i}", tag=f"y{i}")
               for i in range(CT)]

        pair_off = []
        for t0, t1 in PAIRS:
            o0 = _toff(1 + TAPS[t0][0], TAPS[t0][1], TAPS[t0][2])
            if t1 is not None:
                d = _toff(1 + TAPS[t1][0], TAPS[t1][1], TAPS[t1][2]) - o0
            else:
                d = 2  # dead pair slot (zero weights), even stride
            pair_off.append((o0, d))
        assert all(d > 0 and d % 2 == 0 for _, d in pair_off)

        with (tc.tile_pool(name="hpadp", bufs=3) as hpad_pool,
              tc.tile_pool(name="wi8p", bufs=3) as wi8p,
              tc.tile_pool(name="d8p", bufs=3) as d8p,
              tc.tile_pool(name="glueG", bufs=2) as glueG,
              tc.tile_pool(name="wips", bufs=2, space="PSUM") as wips,
              tc.tile_pool(name="cvp2", bufs=3, space="PSUM") as cvp2):

            nmlp = [0]

            def mlp_tile(tt, sink):
                """Wi (fp8 DR) -> staged padded h8 -> conv (fp8 DR pairs);
                sink(nb, ps) consumes each conv psum block immediately."""
                wt = wi8p.tile([128, 3 * 256], FP8, name="wi8t", tag="wi8t")
                nc.sync.dma_start(wt[:], wi8[tt, :, :])
                w4 = wt.rearrange("p (q j m) -> p q j m", q=3, j=2, m=128)
                hp = hpad_pool.tile([128, HPN], FP8, name="hpad", tag="hpad")
                if nmlp[0] < 3:
                    nc.gpsimd.memset(hp[:], 0.0)
                nmlp[0] += 1
                for z in range(Z5):
                    for yh in range(2):
                        ps = wips.tile([128, NB], F32, name="wi_ps",
                                       tag="wi_ps")
                        for q in range(3):
                            mv_ = bass.AP(
                                tensor=xln8[q][:].tensor,
                                offset=(xln8[q][:].offset + z * PL + yh * NB),
                                ap=[xln8[q][:].ap[0], [W5, 2], [1, NB]])
                            nc.tensor.matmul(ps[:], w4[:, q], mv_,
                                             start=(q == 0), stop=(q == 2),
                                             perf_mode=DR)
                        dst = bass.AP(
                            tensor=hp[:].tensor,
                            offset=hp[:].offset + HDR + z * PPL + yh * 312,
                            ap=[hp[:].ap[0], [26, 12], [1, 24]])
                        nc.scalar.activation(dst, ps[:], Copy, scale=1.0 / WSC)
                dgt = d8p.tile([128, NPR * 256], FP8, name="d8t", tag="d8t")
                nc.sync.dma_start(dgt[:], d8[tt, :, :])
                dg4 = dgt.rearrange("p (r j m) -> p r j m", r=NPR, j=2, m=128)
                for o in range(ZP):
                    for bh in range(2):
                        # stream full padded rows (312 wide) so the moving AP
                        # stays 3-dim; consumer reads interior cells strided
                        ps = cvp2.tile([128, 312], F32, name="cv2", tag="cv2")
                        base = hp[:].offset + HDR + (o + 1) * PPL + bh * 312
                        for pr in range(NPR):
                            o0, dlt = pair_off[pr]
                            sv = bass.AP(
                                tensor=hp[:].tensor, offset=base + o0 - PPL,
                                ap=[hp[:].ap[0], [dlt, 2], [1, 312]])
                            nc.tensor.matmul(ps[:], dg4[:, pr], sv,
                                             start=(pr == 0),
                                             stop=(pr == NPR - 1),
                                             perf_mode=DR)
                        sink(o * 2 + bh, ps)

            for u in range(HH):
                def _interior(ps):
                    return bass.AP(tensor=ps[:].tensor, offset=ps[:].offset,
                                   ap=[ps[:].ap[0], [26, 12], [1, 24]])

                g1 = glueG.tile([128, VC], BF16, name="gelu1", tag="gelu1")
                mlp_tile(u, lambda nb, ps: nc.scalar.activation(
                    g1[:, NB * nb:NB * (nb + 1)], _interior(ps), Gelu,
                    scale=1.0 / WSC))
                c2 = glueG.tile([128, VC], BF16, name="conv2", tag="conv2")
                mlp_tile(u + HH, lambda nb, ps: nc.scalar.activation(
                    c2[:, NB * nb:NB * (nb + 1)], _interior(ps), Copy,
                    scale=1.0 / WSC))
                q, j = u // 2, u % 2
                nc.vector.scalar_tensor_tensor(
                    gate8[q][:, j * VC:(j + 1) * VC], g1[:], GSC, c2[:],
                    op0=MULT, op1=MULT)

        with (tc.tile_pool(name="wo8p", bufs=2) as wo8p,
              tc.tile_pool(name="wops", bufs=2, space="PSUM") as wops):
            CH3 = [(0, 512), (512, 512), (1024, 512), (1536, 192)]
            for m in range(CT):
                wt = wo8p.tile([128, 6 * 256], FP8, name="wo8t", tag="wo8t")
                nc.sync.dma_start(wt[:], wo8[m, :, :])
                w4 = wt.rearrange("p (q j m) -> p q j m", q=6, j=2, m=128)
                for o0, n in CH3:
                    ps = wops.tile([128, 512], F32, name="wo_ps", tag="wo_ps")
                    for q in range(6):
                        mv_ = bass.AP(
                            tensor=gate8[q][:].tensor,
                            offset=gate8[q][:].offset + o0,
                            ap=[gate8[q][:].ap[0], [VC, 2], [1, n]])
                        nc.tensor.matmul(ps[:, 0:n], w4[:, q], mv_,
                                         start=(q == 0), stop=(q == 5),
                                         perf_mode=DR)
                    nc.vector.scalar_tensor_tensor(
                        y_t[m][:, o0:o0 + n], ps[:, 0:n],
                        1.0 / (WSC * GSC), xb[m][:, PL + o0:PL + o0 + n],
                        op0=MULT, op1=ADD)

        if DBG:
            with tc.tile_pool(name="dbg2p", bufs=1) as dbg2p:
                for q in range(3):
                    t = dbg2p.tile([128, 2 * W5], F32, name="dbgx", tag="dbgx")
                    nc.scalar.activation(t[:], xln8[q][:], Copy)
                    nc.gpsimd.dma_start(dbg_xln[128 * q:128 * (q + 1), :], t[:])
                for q in range(6):
                    t = dbg2p.tile([128, 2 * VC], F32, name="dbgg", tag="dbgg")
                    nc.scalar.activation(t[:], gate8[q][:], Copy)
                    nc.gpsimd.dma_start(dbg_gate[128 * q:128 * (q + 1), :], t[:])
                for i in range(CT):
                    t = dbg2p.tile([128, VC], F32, name="dbgy", tag="dbgy")
                    nc.scalar.activation(t[:], y_t[i][:], Copy)
                    nc.gpsimd.dma_start(dbg_y[128 * i:128 * (i + 1), :], t[:])

        # =================== LN3 + output ==================================
        with tc.tile_pool(name="ln3bp", bufs=1) as ln3bp:
            muB3, rsB3 = ln_rows(ln3bp, y_t, VC, "ln3")
            with tc.tile_pool(name="glueH", bufs=2) as glueH:
                for i in range(CT):
                    t1 = glueH.tile([128, VC], BF16, name="ln3t", tag="ln3t")
                    nc.vector.tensor_sub(t1[:], y_t[i][:], muB3[:])
                    nc.vector.tensor_mul(t1[:], t1[:], rsB3[:])
                    of = glueH.tile([128, VC], F32, name="outf", tag="outf")
                    nc.vector.tensor_scalar(of[:], t1[:],
                                            sm[i][:, S_N3W:S_N3W + 1],
                                            sm[i][:, S_N3B:S_N3B + 1],
                                            op0=MULT, op1=ADD)
                    nc.gpsimd.dma_start(out[128 * i:128 * (i + 1), :], of[:])

    nc.compile()
    return nc


def _prep(inputs):
    bf = lambda a: np.ascontiguousarray(a).astype(BF)
    f32 = lambda a: np.ascontiguousarray(a, dtype=np.float32)
    fp8 = lambda a: np.ascontiguousarray(a).astype(E4)
    x = f32(inputs["x"][0])
    xf = x.reshape(C, S, PL)

    W_B = f32(inputs["tp_W"]) @ f32(inputs["lora_W2"])
    b_B = f32(inputs["tp_W"]) @ f32(inputs["lora_b2"]) + f32(inputs["tp_b"])
    W_C = f32(inputs["attn_Wo"]) @ f32(inputs["attn_Wv"])
    b_C = f32(inputs["attn_Wo"]) @ f32(inputs["attn_bv"]) + f32(inputs["attn_bo"])

    gind6 = np.zeros((CT, 128, G), np.float32)
    for j in range(CT):
        for p in range(128):
            gind6[j, p, (128 * j + p) // GD] = 1.0
    gexpT = np.zeros((G, C), np.float32)
    for c in range(C):
        gexpT[c // GD, c] = 1.0

    kn_W2 = f32(inputs["kn_W2"])
    kn_W1 = f32(inputs["kn_W1"])
    mlp_Wi = f32(inputs["mlp_Wi"])
    mlp_dw = f32(inputs["mlp_dw"]).reshape(HID, KK)
    mlp_Wo = f32(inputs["mlp_Wo"])

    wi8a = np.zeros((HT, 128, 3 * 256), np.float32)
    WiT = mlp_Wi.T
    for h in range(HT):
        blk = WiT[:, 128 * h:128 * (h + 1)]
        for q in range(3):
            for j in range(2):
                wi8a[h, :, q * 256 + j * 128:q * 256 + j * 128 + 128] = \
                    WSC * blk[256 * q + 128 * j:256 * q + 128 * (j + 1), :]
    d8a = np.zeros((HT, 128, NPR * 256), np.float32)
    idx = np.arange(128)
    for h in range(HT):
        for pr, (t0, t1) in enumerate(PAIRS):
            for j, t in enumerate((t0, t1)):
                if t is None:
                    continue
                d8a[h, idx, pr * 256 + j * 128 + idx] = \
                    WSC * mlp_dw[128 * h:128 * (h + 1), t]
    wo8a = np.zeros((CT, 128, 6 * 256), np.float32)
    WoT = mlp_Wo.T
    for m in range(CT):
        blk = WoT[:, 128 * m:128 * (m + 1)]
        for q in range(6):
            for j in range(2):
                wo8a[m, :, q * 256 + j * 128:q * 256 + j * 128 + 128] = \
                    WSC * blk[256 * q + 128 * j:256 * q + 128 * (j + 1), :]

    com = dict(
        gind6=gind6, gexpT=gexpT,
        ident=bf(np.eye(128, dtype=np.float32)),
        onesc=bf(np.ones((128, 1), np.float32)),
        wAT=bf(f32(inputs["lora_W1"]).T),
        wBT=bf(W_B.T), wCT=bf(W_C.T),
        modWT=bf(f32(inputs["mod_W"]).T),
        w1T=bf(kn_W1.T),
        knb1=f32(inputs["kn_b1"]), knb2r=f32(inputs["kn_b2"]),
        opT=bf(f32(inputs["op_W"]).T),
        wi8=fp8(wi8a), d8=fp8(d8a), wo8=fp8(wo8a),
    )

    n2w = f32(inputs["n2_w"]); n2b = f32(inputs["n2_b"])
    in_maps = []
    for i in range(NCORES):
        z0 = ZP * i
        xh = np.zeros((C, Z7, PL), np.float32)
        lo, hi = max(z0 - 2, 0), min(z0 + ZP + 2, S)
        xh[:, lo - (z0 - 2):lo - (z0 - 2) + (hi - lo)] = xf[:, lo:hi]
        lomask = 1.0 if i > 0 else 0.0
        himask = 1.0 if i < NCORES - 1 else 0.0
        smalls = np.zeros((C, 16), np.float32)
        smalls[:, S_Y] = f32(inputs["y"][0, 0])
        smalls[:, S_BA] = f32(inputs["lora_b1"])
        smalls[:, S_BB] = b_B
        smalls[:, S_BC] = b_C
        smalls[:, S_MODB] = f32(inputs["mod_b"])
        smalls[:, S_OPB] = f32(inputs["op_b"])
        smalls[:, S_N2W] = n2w
        smalls[:, S_N2B] = n2b
        smalls[:, S_N3W] = f32(inputs["n3_w"])
        smalls[:, S_N3B] = f32(inputs["n3_b"])
        smalls[:, S_GNG] = f32(inputs["gn_g"])
        smalls[:, S_GNB] = f32(inputs["gn_b"])
        smalls[:, S_N2WL] = n2w * lomask
        smalls[:, S_N2BL] = n2b * lomask
        smalls[:, S_N2WH] = n2w * himask
        smalls[:, S_N2BH] = n2b * himask
        m = dict(com)
        m.update(
            x_halo=xh.reshape(C, Z7 * PL).astype(BF),
            smalls=smalls,
            w2m=bf(kn_W2[W2R * i:W2R * (i + 1), :].T.reshape(HT, 128, W2R)),
        )
        in_maps.append(m)
    return in_maps


def kernel(**inputs) -> np.ndarray:
    if "nc" not in _CACHE:
        _CACHE["nc"] = build_program()
    nc = _CACHE["nc"]
    in_maps = _prep(inputs)
    res = run_bass_kernel_spmd(nc, in_maps, list(range(NCORES)))
    outs = [res.results[i]["out"].reshape(C, ZP, PL) for i in range(NCORES)]
    full = np.concatenate(outs, axis=1)
    return full.reshape(1, C, S, S, S).astype(np.float32)


# revision 3
# speedup vs baseline: 1.0166x; 1.0166x over previous
"""Bass/Trainium2 SPMD kernel for nn_Block3D (8 NeuronCores) — v2.

z-shard (3 planes/core) with a 2-deep halo (7 input planes) so the LN2
halo exchange is computed locally instead of AllGathered. Collectives are
three small AllGathers (vc partial sums, kernel_net rows, GN stats) with
local reduction. The whole MLP (Wi / depthwise conv / Wo) runs in fp8
DoubleRow (tap-pair matmuls, 4x MAC rate); the CAFM dynamic conv stays
bf16 diag-matmul on PE with DVE-built diagonal stationaries.
"""

from contextlib import ExitStack

import numpy as np
import ml_dtypes

import concourse.bass as bass
import concourse.bacc as bacc
import concourse.tile as tile
from concourse import mybir
from concourse.bass_utils import run_bass_kernel_spmd

BF = ml_dtypes.bfloat16
E4 = ml_dtypes.float8_e4m3fn
F32 = mybir.dt.float32
BF16 = mybir.dt.bfloat16
FP8 = mybir.dt.float8e4

C = 768
G = 12
GD = 64
S = 24
HID = 4 * C          # 3072
HT = HID // 128      # 24
HH = HT // 2         # 12
CT = C // 128        # 6
KK = 27
V = S * S * S
EPS = 1e-5
NCORES = 8
ZP = S // NCORES     # 3 owned planes
PL = S * S           # 576
VC = ZP * PL         # 1728 owned voxels
Z7 = ZP + 4          # 7 input planes
Z5 = ZP + 2          # 5 planes for xb/xln
W5 = Z5 * PL         # 2880
PPL = 26 * 26        # 676 padded plane
HDR = 32             # headroom around padded slabs (keeps offsets even)
MVN = HDR + Z7 * PPL + HDR
HPN = HDR + Z5 * PPL + HDR
KFLAT = C * KK       # 20736
W2R = KFLAT // NCORES  # 2592
NB = 288
WSC = 16.0           # fp8 weight scale
GSC = 256.0          # fp8 gate scale

TAPS = [(dz, dy, dx) for dz in (-1, 0, 1) for dy in (-1, 0, 1) for dx in (-1, 0, 1)]


def _tidx(dz, dy, dx):
    return (dz + 1) * 9 + (dy + 1) * 3 + (dx + 1)


# DoubleRow tap pairs: the pair stride (byte delta between the two moving
# windows) must be EVEN, so pair dx=-1 with dx=+1 (delta 2), and the dx=0
# column across dz/dy (deltas 676 / 26).
PAIRS = ([(_tidx(dz, dy, -1), _tidx(dz, dy, 1))
          for dz in (-1, 0, 1) for dy in (-1, 0, 1)]
         + [(_tidx(-1, dy, 0), _tidx(0, dy, 0)) for dy in (-1, 0, 1)]
         + [(_tidx(1, -1, 0), _tidx(1, 0, 0)), (_tidx(1, 1, 0), None)])
NPR = len(PAIRS)     # 14

_CACHE = {}

Copy = mybir.ActivationFunctionType.Copy
Iden = mybir.ActivationFunctionType.Identity
Gelu = mybir.ActivationFunctionType.Gelu
Sigmoid = mybir.ActivationFunctionType.Sigmoid
Square = mybir.ActivationFunctionType.Square
Sqrt = mybir.ActivationFunctionType.Sqrt
Relu = mybir.ActivationFunctionType.Relu
ADD = mybir.AluOpType.add
SUB = mybir.AluOpType.subtract
MULT = mybir.AluOpType.mult
DR = mybir.MatmulPerfMode.DoubleRow

# smalls columns
(S_Y, S_BA, S_BB, S_BC, S_MODB, S_OPB, S_N2W, S_N2B, S_N3W, S_N3B,
 S_GNG, S_GNB, S_N2WL, S_N2BL, S_N2WH, S_N2BH) = range(16)


def _toff(dz, dy, dx):
    return dz * PPL + dy * 26 + dx


def build_program():
    nc = bacc.Bacc("TRN2", target_bir_lowering=False)

    def dram_in(name, shape, dtype=F32):
        return nc.declare_dram_parameter(name, list(shape), dtype, isOutput=False)

    x_halo = dram_in("x_halo", [C, Z7 * PL], BF16)
    smalls = dram_in("smalls", [C, 16])
    knb1 = dram_in("knb1", [HID])
    knb2r = dram_in("knb2r", [KFLAT])
    gind6 = dram_in("gind6", [CT, 128, G])
    gexpT = dram_in("gexpT", [G, C])
    ident = dram_in("ident", [128, 128], BF16)
    onesc = dram_in("onesc", [128, 1], BF16)
    wAT = dram_in("wAT", [C, C], BF16)
    wBT = dram_in("wBT", [C, C], BF16)
    wCT = dram_in("wCT", [C, C], BF16)
    modWT = dram_in("modWT", [2 * C, C], BF16)
    w1T = dram_in("w1T", [2 * C, HID], BF16)
    w2m = dram_in("w2m", [HT, 128, W2R], BF16)
    opT = dram_in("opT", [C, C], BF16)
    wi8 = dram_in("wi8", [HT, 128, 3 * 256], FP8)
    d8 = dram_in("d8", [HT, 128, NPR * 256], FP8)
    wo8 = dram_in("wo8", [CT, 128, 6 * 256], FP8)
    out = nc.declare_dram_parameter("out", [C, VC], F32, isOutput=True)
    import os as _os
    DBG = _os.environ.get("BLK3D_DBG") == "1"
    if DBG:
        dbg_mod = nc.declare_dram_parameter("dbg_mod", [C, 1], F32, isOutput=True)
        dbg_kern = nc.declare_dram_parameter("dbg_kern", [C, KK], F32, isOutput=True)
        dbg_dyn = nc.declare_dram_parameter("dbg_dyn", [C, W5], F32, isOutput=True)
        dbg_xb = nc.declare_dram_parameter("dbg_xb", [C, W5], F32, isOutput=True)
        dbg_xln = nc.declare_dram_parameter("dbg_xln", [3 * 128, 2 * W5], F32, isOutput=True)
        dbg_gate = nc.declare_dram_parameter("dbg_gate", [6 * 128, 2 * VC], F32, isOutput=True)
        dbg_y = nc.declare_dram_parameter("dbg_y", [C, VC], F32, isOutput=True)
        dbg_comb = nc.declare_dram_parameter("dbg_comb", [C, 2], F32, isOutput=True)
        dbg_kp1 = nc.declare_dram_parameter("dbg_kp1", [HID, 1], F32, isOutput=True)
        dbg_krow = nc.declare_dram_parameter("dbg_krow", [1, W2R], F32, isOutput=True)

    with tile.TileContext(nc) as tc, ExitStack() as ctx:
        dram = ctx.enter_context(tc.tile_pool(name="dram", bufs=1, space="DRAM"))
        persist = ctx.enter_context(tc.tile_pool(name="persist", bufs=1))
        gpool = ctx.enter_context(tc.tile_pool(name="gemv", bufs=2))
        psA = ctx.enter_context(tc.tile_pool(name="psA", bufs=2, space="PSUM"))

        # ---------------- persistent small tiles ----------------
        sm = [persist.tile([128, 16], F32, name=f"sm{i}", tag=f"sm{i}")
              for i in range(CT)]
        for i in range(CT):
            nc.sync.dma_start(sm[i][:], smalls[128 * i:128 * (i + 1), :])
        id_t = persist.tile([128, 128], BF16, name="identt", tag="identt")
        nc.sync.dma_start(id_t[:], ident[:, :])
        ones_t = persist.tile([128, 1], BF16, name="onest", tag="onest")
        nc.sync.dma_start(ones_t[:], onesc[:, :])
        eps_t = persist.tile([128, 1], F32, name="epst", tag="epst")
        nc.vector.memset(eps_t[:], EPS)
        junk = persist.tile([128, VC], BF16, name="junk", tag="junk")
        kern = [persist.tile([128, KK], F32, name=f"kern{i}", tag=f"kern{i}")
                for i in range(CT)]
        cb_cols = [persist.tile([128, 1], F32, name=f"cbc{m}", tag=f"cbc{m}")
                   for m in range(CT)]

        xbp = ctx.enter_context(tc.tile_pool(name="xbp", bufs=1))
        xb = [xbp.tile([128, W5], BF16, name=f"xb{i}", tag=f"xb{i}")
              for i in range(CT)]

        def ln_rows(pool, tiles, width, tag):
            """per-voxel mean/rstd over C -> bf16 bcast tiles [128, width]"""
            with (tc.tile_pool(name=f"{tag}ps", bufs=2, space="PSUM") as lps,
                  tc.tile_pool(name=f"{tag}sq", bufs=2) as sqp,
                  tc.tile_pool(name=f"{tag}rw", bufs=1) as rwp):
                nch = (width + 511) // 512
                row = rwp.tile([1, 2 * width], F32, name="row", tag="row")
                for cidx in range(nch):
                    o0 = 512 * cidx
                    n = min(512, width - o0)
                    ps1 = lps.tile([1, 512], F32, name="s1", tag="s1")
                    for k in range(CT):
                        nc.tensor.matmul(ps1[:, 0:n], ones_t[:],
                                         tiles[k][:, o0:o0 + n],
                                         start=(k == 0), stop=(k == CT - 1))
                    nc.scalar.activation(row[:, o0:o0 + n], ps1[:, 0:n], Copy,
                                         scale=1.0 / C)
                for cidx in range(nch):
                    o0 = 512 * cidx
                    n = min(512, width - o0)
                    ps2 = lps.tile([1, 512], F32, name="s2", tag="s2")
                    for k in range(CT):
                        sq = sqp.tile([128, 512], BF16, name="sq", tag="sq")
                        nc.vector.tensor_mul(sq[:, 0:n], tiles[k][:, o0:o0 + n],
                                             tiles[k][:, o0:o0 + n])
                        nc.tensor.matmul(ps2[:, 0:n], ones_t[:], sq[:, 0:n],
                                         start=(k == 0), stop=(k == CT - 1))
                    nc.scalar.activation(row[:, width + o0:width + o0 + n],
                                         ps2[:, 0:n], Copy, scale=1.0 / C)
                # spread each half to [96, w] for cheap elementwise math
                w96 = width // 96
                rs = rwp.tile([96, 2 * w96], F32, name="rs", tag="rs")
                nc.gpsimd.dma_start(rs[:, 0:w96], row[:, 0:width])
                nc.gpsimd.dma_start(rs[:, w96:2 * w96], row[:, width:2 * width])
                m2 = rwp.tile([96, w96], F32, name="m2", tag="m2")
                nc.scalar.square(m2[:], rs[:, 0:w96])
                vr = rwp.tile([96, w96], F32, name="vr", tag="vr")
                nc.vector.tensor_sub(vr[:], rs[:, w96:2 * w96], m2[:])
                nc.scalar.activation(vr[:], vr[:], Sqrt, bias=eps_t[0:96, 0:1])
                nc.vector.reciprocal(vr[:], vr[:])
                mrow = rwp.tile([1, 2 * width], BF16, name="mrow", tag="mrow")
                nc.gpsimd.dma_start(mrow[:, 0:width], rs[:, 0:w96])
                nc.gpsimd.dma_start(mrow[:, width:2 * width], vr[:])
                muB = pool.tile([128, width], BF16, name=f"{tag}mu",
                                tag=f"{tag}mu")
                rsB = pool.tile([128, width], BF16, name=f"{tag}rs",
                                tag=f"{tag}rs")
                nc.gpsimd.partition_broadcast(muB[:], mrow[0:1, 0:width])
                nc.gpsimd.partition_broadcast(rsB[:], mrow[0:1, width:2 * width])
            return muB, rsB

        # =================== phases A-C: scoped (xs/mv/dyn die after) ======
        with (tc.tile_pool(name="xsp", bufs=1) as xsp,
              tc.tile_pool(name="dynp", bufs=1) as dynp,
              tc.tile_pool(name="opTp", bufs=1) as opTp):
            xs = [xsp.tile([128, Z7 * PL], BF16, name=f"xs{i}", tag=f"xs{i}")
                  for i in range(CT)]
            for i in range(CT):
                nc.sync.dma_start(xs[i][:], x_halo[128 * i:128 * (i + 1), :])
            dyn = [dynp.tile([128, W5], BF16, name=f"dyn{i}", tag=f"dyn{i}")
                   for i in range(CT)]
            opT_t = [opTp.tile([128, C], BF16, name=f"opT{i}", tag=f"opT{i}")
                     for i in range(CT)]
            for i in range(CT):
                nc.sync.dma_start(opT_t[i][:], opT[128 * i:128 * (i + 1), :])

            # ---- vc partials + AG1 ----
            vcs = persist.tile([128, CT], F32, name="vcs", tag="vcs")
            for i in range(CT):
                nc.scalar.activation(junk[:], xs[i][:, 2 * PL:2 * PL + VC],
                                     Copy, accum_out=vcs[:, i:i + 1])
            ag1i = dram.tile([C], F32, name="ag1i", tag="ag1i")
            ag1o = dram.tile([NCORES, C], F32, name="ag1o", tag="ag1o",
                             addr_space="Shared")
            nc.gpsimd.dma_start(
                bass.AP(tensor=ag1i[:].tensor, offset=ag1i[:].offset,
                        ap=[[1, 128], [128, CT]]), vcs[:])
            nc.gpsimd.collective_compute(
                "AllGather", mybir.AluOpType.bypass,
                replica_groups=[list(range(NCORES))], ins=[ag1i[:]],
                outs=[ag1o[:]])

            # ---- text chain / mod / kp1 (row-form gemv) ----
            _psRs = ExitStack()
            psR = _psRs.enter_context(
                tc.tile_pool(name="psR", bufs=1, space="PSUM"))
            pcs = [psR.tile([1, 512], F32, name=f"rp{j}", tag=f"rowps{j}")
                   for j in range(6)]
            with tc.tile_pool(name="wstream", bufs=2) as wpool:

                def gemv_chain(wdram, in_cols, nk, nm, act, bias_col, tag,
                               odt=BF16, bias_t=None):
                    # out = W @ in  via moving-weights row matmuls; the row
                    # [1, 128*nm] is transposed to columns through DRAM.
                    width = 128 * nm
                    npc = (width + 511) // 512
                    for k in range(nk):
                        wt = wpool.tile([128, width], BF16, name=f"{tag}w",
                                        tag=f"{tag}w")
                        nc.sync.dma_start(wt[:],
                                          wdram[128 * k:128 * (k + 1), :])
                        for pc in range(npc):
                            o0 = 512 * pc
                            n = min(512, width - o0)
                            nc.tensor.matmul(pcs[pc][:, 0:n], in_cols[k][:],
                                             wt[:, o0:o0 + n],
                                             start=(k == 0),
                                             stop=(k == nk - 1))
                    row = gpool.tile([1, width], BF16, name=f"{tag}row",
                                     tag=f"{tag}row")
                    for pc in range(npc):
                        o0 = 512 * pc
                        n = min(512, width - o0)
                        nc.scalar.activation(row[:, o0:o0 + n],
                                             pcs[pc][:, 0:n], Copy)
                    drow = dram.tile([width], BF16, name=f"{tag}dr",
                                     tag=f"{tag}dr")
                    nc.gpsimd.dma_start(drow[:], row[:])
                    ct = gpool.tile([128, nm], BF16, name=f"{tag}ct",
                                    tag=f"{tag}ct")
                    nc.gpsimd.dma_start(
                        ct[:], bass.AP(tensor=drow[:].tensor,
                                       offset=drow[:].offset,
                                       ap=[[1, 128], [128, nm]]))
                    outs = []
                    for m in range(nm):
                        if bias_t is not None:
                            bias = bias_t[:, m:m + 1]
                        else:
                            bias = sm[m][:, bias_col:bias_col + 1]
                        o = gpool.tile([128, 1], odt, name=f"{tag}o{m}",
                                       tag=f"{tag}o{m}")
                        nc.scalar.activation(o[:], ct[:, m:m + 1], act,
                                             bias=bias)
                        outs.append(o)
                    return outs

                y_cols = []
                for i in range(CT):
                    t = gpool.tile([128, 1], BF16, name=f"yc{i}", tag=f"yc{i}")
                    nc.scalar.activation(t[:], sm[i][:, S_Y:S_Y + 1], Copy)
                    y_cols.append(t)
                hA = gemv_chain(wAT, y_cols, CT, CT, Relu, S_BA, "wa")
                hB = gemv_chain(wBT, hA, CT, CT, Iden, S_BB, "wb")
                attn = gemv_chain(wCT, hB, CT, CT, Iden, S_BC, "wc")

                vc8 = persist.tile([128, CT * NCORES], F32, name="vc8",
                                   tag="vc8")
                nc.gpsimd.dma_start(
                    bass.AP(tensor=vc8[:].tensor, offset=vc8[:].offset,
                            ap=[vc8[:].ap[0], [CT, NCORES], [1, CT]]),
                    bass.AP(tensor=ag1o[:].tensor, offset=ag1o[:].offset,
                            ap=[[1, 128], [C, NCORES], [128, CT]]))
                v3 = vc8.rearrange("p (i k) -> p i k", i=NCORES, k=CT)
                for step in (4, 2, 1):
                    nc.vector.tensor_add(v3[:, 0:step, :], v3[:, 0:step, :],
                                         v3[:, step:2 * step, :])
                comb = []
                for i in range(CT):
                    cb = gpool.tile([128, 1], BF16, name=f"cmb{i}",
                                    tag=f"cmb{i}")
                    nc.scalar.activation(cb[:], vc8[:, i:i + 1], Copy,
                                         scale=1.0 / V)
                    comb.append(cb)
                comb += attn

                mod = gemv_chain(modWT, comb, 2 * CT, CT, Sigmoid, S_MODB,
                                 "md", odt=F32)

                knb1_t = persist.tile([128, HT], F32, name="knb1t",
                                      tag="knb1t")
                nc.gpsimd.dma_start(
                    knb1_t[:],
                    bass.AP(tensor=knb1, offset=0, ap=[[1, 128], [128, HT]]))
                kp1 = gemv_chain(w1T, comb, 2 * CT, HT, Relu, None, "k1",
                                 bias_t=knb1_t)

            if DBG:
                with tc.tile_pool(name="dbg0p", bufs=2) as dbg0p:
                    for i in range(CT):
                        t = dbg0p.tile([128, 2], F32, name="dbgc", tag="dbgc")
                        nc.scalar.activation(t[:, 0:1], comb[i][:], Copy)
                        nc.scalar.activation(t[:, 1:2], comb[CT + i][:], Copy)
                        nc.gpsimd.dma_start(dbg_comb[128 * i:128 * (i + 1), :],
                                            t[:])
                    for m in range(HT):
                        t = dbg0p.tile([128, 1], F32, name="dbgk1", tag="dbgk1")
                        nc.scalar.activation(t[:], kp1[m][:], Copy)
                        nc.gpsimd.dma_start(
                            dbg_kp1[128 * m:128 * (m + 1), :], t[:])

            # ---- W2 own rows + AG2 -> kernels ----
            ag2i = dram.tile([W2R], F32, name="ag2i", tag="ag2i")
            ag2o = dram.tile([NCORES, W2R], F32, name="ag2o", tag="ag2o",
                             addr_space="Shared")
            PIECES = [(0, 512), (512, 512), (1024, 512), (1536, 512),
                      (2048, 512), (2560, 32)]
            with (tc.tile_pool(name="w2s", bufs=3) as w2s,
                  tc.tile_pool(name="krowp", bufs=1) as krowp):
                pcs = [psR.tile([1, 512], F32, name=f"w2p{j}",
                                tag=f"rowps{j}") for j in range(6)]
                for k in range(HT):
                    wt = w2s.tile([128, W2R], BF16, name="w2w", tag="w2w")
                    nc.sync.dma_start(wt[:], w2m[k, :, :])
                    for j, (o0, n) in enumerate(PIECES):
                        nc.tensor.matmul(pcs[j][:, 0:n], kp1[k][:],
                                         wt[:, o0:o0 + n], start=(k == 0),
                                         stop=(k == HT - 1))
                krow = krowp.tile([1, W2R], F32, name="krow", tag="krow")
                for j, (o0, n) in enumerate(PIECES):
                    nc.scalar.activation(krow[:, o0:o0 + n], pcs[j][:, 0:n],
                                         Copy)
                nc.gpsimd.dma_start(ag2i[:], krow[:])
                if DBG:
                    nc.gpsimd.dma_start(dbg_krow[:, :], krow[:])
            nc.gpsimd.collective_compute(
                "AllGather", mybir.AluOpType.bypass,
                replica_groups=[list(range(NCORES))], ins=[ag2i[:]],
                outs=[ag2o[:]])
            for i in range(CT):
                nc.gpsimd.dma_start(
                    kern[i][:],
                    bass.AP(tensor=ag2o[:].tensor,
                            offset=ag2o[:].offset + 128 * i * KK,
                            ap=[[KK, 128], [1, KK]]))
                kb = gpool.tile([128, KK], F32, name="kernb", tag="kernb")
                nc.gpsimd.dma_start(
                    kb[:], bass.AP(tensor=knb2r, offset=128 * i * KK,
                                   ap=[[KK, 128], [1, KK]]))
                nc.vector.tensor_add(kern[i][:], kern[i][:], kb[:])
            _psRs.close()

            # ---- mv staging + dyn conv + GN stats ----
            ag3i = dram.tile([G, 2], F32, name="ag3i", tag="ag3i")
            ag3o = dram.tile([NCORES, G, 2], F32, name="ag3o", tag="ag3o",
                             addr_space="Shared")
            with (tc.tile_pool(name="mvp", bufs=2) as mvp,
                  tc.tile_pool(name="diagp", bufs=2) as diagp,
                  tc.tile_pool(name="cvps", bufs=4, space="PSUM") as cvps,
                  tc.tile_pool(name="gnps", bufs=1, space="PSUM") as gnps,
                  tc.tile_pool(name="gnst", bufs=2) as gnst):
                gps = gnps.tile([G, 2], F32, name="gps", tag="gps")
                for i in range(CT):
                    mv = mvp.tile([128, MVN], BF16, name="mv", tag="mv")
                    if i < 2:
                        nc.gpsimd.memset(mv[:], 0.0)
                    for z in range(Z7):
                        for yh in range(2):
                            dst = bass.AP(
                                tensor=mv[:].tensor,
                                offset=(mv[:].offset + HDR + z * PPL
                                        + yh * 312),
                                ap=[mv[:].ap[0], [26, 12], [1, 24]])
                            nc.vector.tensor_scalar_mul(
                                dst,
                                xs[i][:, z * PL + yh * NB:
                                      z * PL + yh * NB + NB],
                                mod[i][:, 0:1])
                    dg = [diagp.tile([128, 128], BF16, name=f"dg{t}",
                                     tag=f"dg{t}") for t in range(KK)]
                    for t in range(KK):
                        nc.vector.tensor_scalar_mul(dg[t][:], id_t[:],
                                                    kern[i][:, t:t + 1])
                    for o in range(Z5):
                        for bh in range(2):
                            ps = cvps.tile([128, NB], F32, name="cv",
                                           tag="cv")
                            base = (mv[:].offset + HDR + o * PPL + bh * 312)
                            for t, (dz, dy, dx) in enumerate(TAPS):
                                sv = bass.AP(
                                    tensor=mv[:].tensor,
                                    offset=base + _toff(1 + dz, dy, dx),
                                    ap=[mv[:].ap[0], [26, 12], [1, 24]])
                                nc.tensor.matmul(ps[:], dg[t][:], sv,
                                                 start=(t == 0),
                                                 stop=(t == KK - 1))
                            nc.scalar.activation(
                                dyn[i][:, o * PL + bh * NB:
                                       o * PL + bh * NB + NB],
                                ps[:], Copy)
                    st = gnst.tile([128, 2], F32, name="gnstat", tag="gnstat")
                    nc.scalar.activation(junk[:], dyn[i][:, PL:PL + VC], Copy,
                                         accum_out=st[:, 0:1])
                    nc.scalar.activation(junk[:], dyn[i][:, PL:PL + VC],
                                         Square, accum_out=st[:, 1:2])
                    gi = gnst.tile([128, G], F32, name="gind", tag="gind")
                    nc.gpsimd.dma_start(gi[:], gind6[i, :, :])
                    nc.tensor.matmul(gps[:], gi[:], st[:], start=(i == 0),
                                     stop=(i == CT - 1))
                gsb = persist.tile([G, 2], F32, name="gsb", tag="gsb")
                nc.scalar.activation(gsb[:], gps[:], Copy)
            nc.gpsimd.dma_start(ag3i[:], gsb[:])
            nc.gpsimd.collective_compute(
                "AllGather", mybir.AluOpType.bypass,
                replica_groups=[list(range(NCORES))], ins=[ag3i[:]],
                outs=[ag3o[:]])

            gst8 = persist.tile([G, 2 * NCORES], F32, name="gst8", tag="gst8")
            nc.gpsimd.dma_start(
                bass.AP(tensor=gst8[:].tensor, offset=gst8[:].offset,
                        ap=[gst8[:].ap[0], [2, NCORES], [1, 2]]),
                bass.AP(tensor=ag3o[:].tensor, offset=ag3o[:].offset,
                        ap=[[2, G], [2 * G, NCORES], [1, 2]]))
            g3 = gst8.rearrange("p (i t) -> p i t", i=NCORES, t=2)
            for step in (4, 2, 1):
                nc.vector.tensor_add(g3[:, 0:step, :], g3[:, 0:step, :],
                                     g3[:, step:2 * step, :])
            NGRP = float(GD * V)
            gmr = persist.tile([G, 2], F32, name="gmr", tag="gmr")
            nc.scalar.activation(gmr[:, 0:1], gst8[:, 0:1], Copy,
                                 scale=1.0 / NGRP)
            musq = persist.tile([G, 1], F32, name="musq", tag="musq")
            nc.scalar.square(musq[:], gmr[:, 0:1])
            var = persist.tile([G, 1], F32, name="gvar", tag="gvar")
            nc.vector.tensor_scalar(var[:], gst8[:, 1:2], 1.0 / NGRP, None,
                                    op0=MULT)
            nc.vector.tensor_sub(var[:], var[:], musq[:])
            nc.scalar.activation(var[:], var[:], Sqrt, bias=eps_t[0:G, 0:1])
            nc.vector.reciprocal(gmr[:, 1:2], var[:])

            with tc.tile_pool(name="gnf", bufs=2) as gnf:
                shifts = []
                gscs = []
                for i in range(CT):
                    ge = gnf.tile([G, 128], F32, name=f"gexp{i}",
                                  tag=f"gexp{i}")
                    nc.gpsimd.dma_start(ge[:], gexpT[:, 128 * i:128 * (i + 1)])
                    ps = psA.tile([128, 2], F32, name="gn2", tag="gvps")
                    nc.tensor.matmul(ps[:], ge[:], gmr[:], start=True,
                                     stop=True)
                    mu_c = gnf.tile([128, 2], F32, name=f"muc{i}",
                                    tag=f"muc{i}")
                    nc.scalar.activation(mu_c[:], ps[:], Copy)
                    a = persist.tile([128, 1], F32, name=f"gsc{i}",
                                     tag=f"gsc{i}")
                    nc.vector.tensor_mul(a[:], sm[i][:, S_GNG:S_GNG + 1],
                                         mu_c[:, 1:2])
                    b = gnf.tile([128, 1], BF16, name=f"gsh{i}", tag=f"gsh{i}")
                    t = gnf.tile([128, 1], F32, name="gtmp", tag="gtmp")
                    nc.vector.tensor_mul(t[:], mu_c[:, 0:1], a[:])
                    nc.vector.tensor_sub(t[:], sm[i][:, S_GNB:S_GNB + 1], t[:])
                    nc.scalar.activation(b[:], t[:], Copy)
                    shifts.append(b)
                    gscs.append(a)
                for m in range(CT):
                    ps = psA.tile([128, 1], F32, name="cbp", tag="gvps")
                    for k in range(CT):
                        nc.tensor.matmul(ps[:],
                                         opT_t[k][:, 128 * m:128 * (m + 1)],
                                         shifts[k][:], start=(k == 0),
                                         stop=(k == CT - 1))
                    nc.scalar.activation(cb_cols[m][:], ps[:], Iden,
                                         bias=sm[m][:, S_OPB:S_OPB + 1])
                for i in range(CT):
                    nc.vector.tensor_scalar_mul(opT_t[i][:], opT_t[i][:],
                                                gscs[i][:])

            # cafm matmul (5 planes) + xb = (psum + cb) * x
            CH5 = [(0, 512), (512, 512), (1024, 512), (1536, 512),
                   (2048, 512), (2560, 320)]
            with tc.tile_pool(name="opwps", bufs=2, space="PSUM") as opwps:
                for m in range(CT):
                    for o0, n in CH5:
                        ps = opwps.tile([128, 512], F32, name="opw", tag="opw")
                        for k in range(CT):
                            nc.tensor.matmul(
                                ps[:, 0:n],
                                opT_t[k][:, 128 * m:128 * (m + 1)],
                                dyn[k][:, o0:o0 + n], start=(k == 0),
                                stop=(k == CT - 1))
                        nc.vector.scalar_tensor_tensor(
                            xb[m][:, o0:o0 + n], ps[:, 0:n], cb_cols[m][:],
                            xs[m][:, PL + o0:PL + o0 + n], op0=ADD, op1=MULT)

            if DBG:
                with tc.tile_pool(name="dbgp", bufs=2) as dbgp:
                    for i in range(CT):
                        t = dbgp.tile([128, W5], F32, name="dbgt", tag="dbgt")
                        nc.scalar.activation(t[:], dyn[i][:], Copy)
                        nc.gpsimd.dma_start(dbg_dyn[128 * i:128 * (i + 1), :],
                                            t[:])
                        t2 = dbgp.tile([128, W5], F32, name="dbgt2",
                                       tag="dbgt2")
                        nc.scalar.activation(t2[:], xb[i][:], Copy)
                        nc.gpsimd.dma_start(dbg_xb[128 * i:128 * (i + 1), :],
                                            t2[:])
                        nc.gpsimd.dma_start(dbg_kern[128 * i:128 * (i + 1), :],
                                            kern[i][:])
                        t3 = dbgp.tile([128, 1], F32, name="dbgt3",
                                       tag="dbgt3")
                        nc.scalar.activation(t3[:], mod[i][:], Copy)
                        nc.gpsimd.dma_start(dbg_mod[128 * i:128 * (i + 1), :],
                                            t3[:])

        # =================== LN2 -> xln8 (fp8, channel-paired) =============
        xln8p = ctx.enter_context(tc.tile_pool(name="xln8p", bufs=1))
        xln8 = [xln8p.tile([128, 2 * W5], FP8, name=f"xl{q}", tag=f"xl{q}")
                for q in range(3)]
        with tc.tile_pool(name="lnbp", bufs=1) as lnbp:
            muB, rsB = ln_rows(lnbp, xb, W5, "ln2")
            with tc.tile_pool(name="lnt", bufs=2) as lnt:
                for i in range(CT):
                    t1 = lnt.tile([128, W5], BF16, name="lnt1", tag="lnt1")
                    nc.vector.tensor_sub(t1[:], xb[i][:], muB[:])
                    nc.vector.tensor_mul(t1[:], t1[:], rsB[:])
                    q, j = i // 2, i % 2
                    dst = xln8[q][:, j * W5:(j + 1) * W5]
                    nc.scalar.activation(dst[:, 0:PL], t1[:, 0:PL], Iden,
                                         bias=sm[i][:, S_N2BL:S_N2BL + 1],
                                         scale=sm[i][:, S_N2WL:S_N2WL + 1])
                    nc.scalar.activation(dst[:, PL:4 * PL], t1[:, PL:4 * PL],
                                         Iden,
                                         bias=sm[i][:, S_N2B:S_N2B + 1],
                                         scale=sm[i][:, S_N2W:S_N2W + 1])
                    nc.scalar.activation(dst[:, 4 * PL:5 * PL],
                                         t1[:, 4 * PL:5 * PL], Iden,
                                         bias=sm[i][:, S_N2BH:S_N2BH + 1],
                                         scale=sm[i][:, S_N2WH:S_N2WH + 1])

        # =================== MLP: fp8 DoubleRow ===========================
        gate8p = ctx.enter_context(tc.tile_pool(name="gate8p", bufs=1))
        gate8 = [gate8p.tile([128, 2 * VC], FP8, name=f"g8{q}", tag=f"g8{q}")
                 for q in range(6)]
        ytp = ctx.enter_context(tc.tile_pool(name="ytp", bufs=1))
        y_t = [ytp.tile([128, VC], BF16, name=f"y{i}", tag=f"y{i}")
               for i in range(CT)]

        pair_off = []
        for t0, t1 in PAIRS:
            o0 = _toff(1 + TAPS[t0][0], TAPS[t0][1], TAPS[t0][2])
            if t1 is not None:
                d = _toff(1 + TAPS[t1][0], TAPS[t1][1], TAPS[t1][2]) - o0
            else:
                d = 2  # dead pair slot (zero weights), even stride
            pair_off.append((o0, d))
        assert all(d > 0 and d % 2 == 0 for _, d in pair_off)

        with (tc.tile_pool(name="hpadp", bufs=3) as hpad_pool,
              tc.tile_pool(name="wi8p", bufs=3) as wi8p,
              tc.tile_pool(name="d8p", bufs=3) as d8p,
              tc.tile_pool(name="glueG", bufs=2) as glueG,
              tc.tile_pool(name="wips", bufs=2, space="PSUM") as wips,
              tc.tile_pool(name="cvp2", bufs=3, space="PSUM") as cvp2):

            nmlp = [0]

            def mlp_tile(tt, sink):
                """Wi (fp8 DR) -> staged padded h8 -> conv (fp8 DR pairs);
                sink(nb, ps) consumes each conv psum block immediately."""
                wt = wi8p.tile([128, 3 * 256], FP8, name="wi8t", tag="wi8t")
                nc.sync.dma_start(wt[:], wi8[tt, :, :])
                w4 = wt.rearrange("p (q j m) -> p q j m", q=3, j=2, m=128)
                hp = hpad_pool.tile([128, HPN], FP8, name="hpad", tag="hpad")
                if nmlp[0] < 3:
                    nc.gpsimd.memset(hp[:], 0.0)
                nmlp[0] += 1
                for z in range(Z5):
                    for yh in range(2):
                        ps = wips.tile([128, NB], F32, name="wi_ps",
                                       tag="wi_ps")
                        for q in range(3):
                            mv_ = bass.AP(
                                tensor=xln8[q][:].tensor,
                                offset=(xln8[q][:].offset + z * PL + yh * NB),
                                ap=[xln8[q][:].ap[0], [W5, 2], [1, NB]])
                            nc.tensor.matmul(ps[:], w4[:, q], mv_,
                                             start=(q == 0), stop=(q == 2),
                                             perf_mode=DR)
                        dst = bass.AP(
                            tensor=hp[:].tensor,
                            offset=hp[:].offset + HDR + z * PPL + yh * 312,
                            ap=[hp[:].ap[0], [26, 12], [1, 24]])
                        if z in (1, 3):
                            nc.vector.tensor_scalar_mul(dst, ps[:], 1.0 / WSC)
                        else:
                            nc.scalar.activation(dst, ps[:], Copy,
                                                 scale=1.0 / WSC)
                dgt = d8p.tile([128, NPR * 256], FP8, name="d8t", tag="d8t")
                nc.sync.dma_start(dgt[:], d8[tt, :, :])
                dg4 = dgt.rearrange("p (r j m) -> p r j m", r=NPR, j=2, m=128)
                for o in range(ZP):
                    for bh in range(2):
                        # stream full padded rows (312 wide) so the moving AP
                        # stays 3-dim; consumer reads interior cells strided
                        ps = cvp2.tile([128, 312], F32, name="cv2", tag="cv2")
                        base = hp[:].offset + HDR + (o + 1) * PPL + bh * 312
                        for pr in range(NPR):
                            o0, dlt = pair_off[pr]
                            sv = bass.AP(
                                tensor=hp[:].tensor, offset=base + o0 - PPL,
                                ap=[hp[:].ap[0], [dlt, 2], [1, 312]])
                            nc.tensor.matmul(ps[:], dg4[:, pr], sv,
                                             start=(pr == 0),
                                             stop=(pr == NPR - 1),
                                             perf_mode=DR)
                        sink(o * 2 + bh, ps)

            for u in range(HH):
                def _interior(ps):
                    return bass.AP(tensor=ps[:].tensor, offset=ps[:].offset,
                                   ap=[ps[:].ap[0], [26, 12], [1, 24]])

                g1 = glueG.tile([128, VC], BF16, name="gelu1", tag="gelu1")
                mlp_tile(u, lambda nb, ps: nc.scalar.activation(
                    g1[:, NB * nb:NB * (nb + 1)], _interior(ps), Gelu,
                    scale=1.0 / WSC))
                q, j = u // 2, u % 2
                # gate = (gelu(c1) * GSC/WSC) * conv2, reading conv2 psum
                # blocks directly (scale folded: GSC/WSC applied to g1)
                mlp_tile(u + HH, lambda nb, ps: nc.vector.scalar_tensor_tensor(
                    gate8[q][:, j * VC + NB * nb:j * VC + NB * (nb + 1)],
                    g1[:, NB * nb:NB * (nb + 1)], GSC / WSC, _interior(ps),
                    op0=MULT, op1=MULT))

        with (tc.tile_pool(name="wo8p", bufs=2) as wo8p,
              tc.tile_pool(name="wops", bufs=2, space="PSUM") as wops):
            CH3 = [(0, 512), (512, 512), (1024, 512), (1536, 192)]
            for m in range(CT):
                wt = wo8p.tile([128, 6 * 256], FP8, name="wo8t", tag="wo8t")
                nc.sync.dma_start(wt[:], wo8[m, :, :])
                w4 = wt.rearrange("p (q j m) -> p q j m", q=6, j=2, m=128)
                for o0, n in CH3:
                    ps = wops.tile([128, 512], F32, name="wo_ps", tag="wo_ps")
                    for q in range(6):
                        mv_ = bass.AP(
                            tensor=gate8[q][:].tensor,
                            offset=gate8[q][:].offset + o0,
                            ap=[gate8[q][:].ap[0], [VC, 2], [1, n]])
                        nc.tensor.matmul(ps[:, 0:n], w4[:, q], mv_,
                                         start=(q == 0), stop=(q == 5),
                                         perf_mode=DR)
                    nc.vector.scalar_tensor_tensor(
                        y_t[m][:, o0:o0 + n], ps[:, 0:n],
                        1.0 / (WSC * GSC), xb[m][:, PL + o0:PL + o0 + n],
                        op0=MULT, op1=ADD)

        if DBG:
            with tc.tile_pool(name="dbg2p", bufs=1) as dbg2p:
                for q in range(3):
                    t = dbg2p.tile([128, 2 * W5], F32, name="dbgx", tag="dbgx")
                    nc.scalar.activation(t[:], xln8[q][:], Copy)
                    nc.gpsimd.dma_start(dbg_xln[128 * q:128 * (q + 1), :], t[:])
                for q in range(6):
                    t = dbg2p.tile([128, 2 * VC], F32, name="dbgg", tag="dbgg")
                    nc.scalar.activation(t[:], gate8[q][:], Copy)
                    nc.gpsimd.dma_start(dbg_gate[128 * q:128 * (q + 1), :], t[:])
                for i in range(CT):
                    t = dbg2p.tile([128, VC], F32, name="dbgy", tag="dbgy")
                    nc.scalar.activation(t[:], y_t[i][:], Copy)
                    nc.gpsimd.dma_start(dbg_y[128 * i:128 * (i + 1), :], t[:])

        # =================== LN3 + output ==================================
        with tc.tile_pool(name="ln3bp", bufs=1) as ln3bp:
            muB3, rsB3 = ln_rows(ln3bp, y_t, VC, "ln3")
            with tc.tile_pool(name="glueH", bufs=2) as glueH:
                for i in range(CT):
                    t1 = glueH.tile([128, VC], BF16, name="ln3t", tag="ln3t")
                    nc.vector.tensor_sub(t1[:], y_t[i][:], muB3[:])
                    nc.vector.tensor_mul(t1[:], t1[:], rsB3[:])
                    of = glueH.tile([128, VC], F32, name="outf", tag="outf")
                    nc.scalar.activation(of[:], t1[:], Iden,
                                         bias=sm[i][:, S_N3B:S_N3B + 1],
                                         scale=sm[i][:, S_N3W:S_N3W + 1])
                    nc.gpsimd.dma_start(out[128 * i:128 * (i + 1), :], of[:])

    nc.compile()
    return nc


def _prep(inputs):
    bf = lambda a: np.ascontiguousarray(a).astype(BF)
    f32 = lambda a: np.ascontiguousarray(a, dtype=np.float32)
    fp8 = lambda a: np.ascontiguousarray(a).astype(E4)
    x = f32(inputs["x"][0])
    xf = x.reshape(C, S, PL)

    W_B = f32(inputs["tp_W"]) @ f32(inputs["lora_W2"])
    b_B = f32(inputs["tp_W"]) @ f32(inputs["lora_b2"]) + f32(inputs["tp_b"])
    W_C = f32(inputs["attn_Wo"]) @ f32(inputs["attn_Wv"])
    b_C = f32(inputs["attn_Wo"]) @ f32(inputs["attn_bv"]) + f32(inputs["attn_bo"])

    gind6 = np.zeros((CT, 128, G), np.float32)
    for j in range(CT):
        for p in range(128):
            gind6[j, p, (128 * j + p) // GD] = 1.0
    gexpT = np.zeros((G, C), np.float32)
    for c in range(C):
        gexpT[c // GD, c] = 1.0

    kn_W2 = f32(inputs["kn_W2"])
    kn_W1 = f32(inputs["kn_W1"])
    mlp_Wi = f32(inputs["mlp_Wi"])
    mlp_dw = f32(inputs["mlp_dw"]).reshape(HID, KK)
    mlp_Wo = f32(inputs["mlp_Wo"])

    wi8a = np.zeros((HT, 128, 3 * 256), np.float32)
    WiT = mlp_Wi.T
    for h in range(HT):
        blk = WiT[:, 128 * h:128 * (h + 1)]
        for q in range(3):
            for j in range(2):
                wi8a[h, :, q * 256 + j * 128:q * 256 + j * 128 + 128] = \
                    WSC * blk[256 * q + 128 * j:256 * q + 128 * (j + 1), :]
    d8a = np.zeros((HT, 128, NPR * 256), np.float32)
    idx = np.arange(128)
    for h in range(HT):
        for pr, (t0, t1) in enumerate(PAIRS):
            for j, t in enumerate((t0, t1)):
                if t is None:
                    continue
                d8a[h, idx, pr * 256 + j * 128 + idx] = \
                    WSC * mlp_dw[128 * h:128 * (h + 1), t]
    wo8a = np.zeros((CT, 128, 6 * 256), np.float32)
    WoT = mlp_Wo.T
    for m in range(CT):
        blk = WoT[:, 128 * m:128 * (m + 1)]
        for q in range(6):
            for j in range(2):
                wo8a[m, :, q * 256 + j * 128:q * 256 + j * 128 + 128] = \
                    WSC * blk[256 * q + 128 * j:256 * q + 128 * (j + 1), :]

    com = dict(
        gind6=gind6, gexpT=gexpT,
        ident=bf(np.eye(128, dtype=np.float32)),
        onesc=bf(np.ones((128, 1), np.float32)),
        wAT=bf(f32(inputs["lora_W1"]).T),
        wBT=bf(W_B.T), wCT=bf(W_C.T),
        modWT=bf(f32(inputs["mod_W"]).T),
        w1T=bf(kn_W1.T),
        knb1=f32(inputs["kn_b1"]), knb2r=f32(inputs["kn_b2"]),
        opT=bf(f32(inputs["op_W"]).T),
        wi8=fp8(wi8a), d8=fp8(d8a), wo8=fp8(wo8a),
    )

    n2w = f32(inputs["n2_w"]); n2b = f32(inputs["n2_b"])
    in_maps = []
    for i in range(NCORES):
        z0 = ZP * i
        xh = np.zeros((C, Z7, PL), np.float32)
        lo, hi = max(z0 - 2, 0), min(z0 + ZP + 2, S)
        xh[:, lo - (z0 - 2):lo - (z0 - 2) + (hi - lo)] = xf[:, lo:hi]
        lomask = 1.0 if i > 0 else 0.0
        himask = 1.0 if i < NCORES - 1 else 0.0
        smalls = np.zeros((C, 16), np.float32)
        smalls[:, S_Y] = f32(inputs["y"][0, 0])
        smalls[:, S_BA] = f32(inputs["lora_b1"])
        smalls[:, S_BB] = b_B
        smalls[:, S_BC] = b_C
        smalls[:, S_MODB] = f32(inputs["mod_b"])
        smalls[:, S_OPB] = f32(inputs["op_b"])
        smalls[:, S_N2W] = n2w
        smalls[:, S_N2B] = n2b
        smalls[:, S_N3W] = f32(inputs["n3_w"])
        smalls[:, S_N3B] = f32(inputs["n3_b"])
        smalls[:, S_GNG] = f32(inputs["gn_g"])
        smalls[:, S_GNB] = f32(inputs["gn_b"])
        smalls[:, S_N2WL] = n2w * lomask
        smalls[:, S_N2BL] = n2b * lomask
        smalls[:, S_N2WH] = n2w * himask
        smalls[:, S_N2BH] = n2b * himask
        m = dict(com)
        m.update(
            x_halo=xh.reshape(C, Z7 * PL).astype(BF),
            smalls=smalls,
            w2m=bf(kn_W2[W2R * i:W2R * (i + 1), :].T.reshape(HT, 128, W2R)),
        )
        in_maps.append(m)
    return in_maps


def kernel(**inputs) -> np.ndarray:
    if "nc" not in _CACHE:
        _CACHE["nc"] = build_program()
    nc = _CACHE["nc"]
    in_maps = _prep(inputs)
    res = run_bass_kernel_spmd(nc, in_maps, list(range(NCORES)))
    outs = [res.results[i]["out"].reshape(C, ZP, PL) for i in range(NCORES)]
    full = np.concatenate(outs, axis=1)
    return full.reshape(1, C, S, S, S).astype(np.float32)


# revision 4
# speedup vs baseline: 1.0534x; 1.0362x over previous
"""Bass/Trainium2 SPMD kernel for nn_Block3D (8 NeuronCores) — v2.

z-shard (3 planes/core) with a 2-deep halo (7 input planes) so the LN2
halo exchange is computed locally instead of AllGathered. Collectives are
three small AllGathers (vc partial sums, kernel_net rows, GN stats) with
local reduction. The whole MLP (Wi / depthwise conv / Wo) runs in fp8
DoubleRow (tap-pair matmuls, 4x MAC rate); the CAFM dynamic conv stays
bf16 diag-matmul on PE with DVE-built diagonal stationaries.
"""

from contextlib import ExitStack

import numpy as np
import ml_dtypes

import concourse.bass as bass
import concourse.bacc as bacc
import concourse.tile as tile
from concourse import mybir
from concourse.bass_utils import run_bass_kernel_spmd

BF = ml_dtypes.bfloat16
E4 = ml_dtypes.float8_e4m3fn
F32 = mybir.dt.float32
BF16 = mybir.dt.bfloat16
FP8 = mybir.dt.float8e4

C = 768
G = 12
GD = 64
S = 24
HID = 4 * C          # 3072
HT = HID // 128      # 24
HH = HT // 2         # 12
CT = C // 128        # 6
KK = 27
V = S * S * S
EPS = 1e-5
NCORES = 8
ZP = S // NCORES     # 3 owned planes
PL = S * S           # 576
VC = ZP * PL         # 1728 owned voxels
Z7 = ZP + 4          # 7 input planes
Z5 = ZP + 2          # 5 planes for xb/xln
W5 = Z5 * PL         # 2880
PPL = 26 * 26        # 676 padded plane
HDR = 32             # headroom around padded slabs (keeps offsets even)
MVN = HDR + Z7 * PPL + HDR
HPN = HDR + Z5 * PPL + HDR
KFLAT = C * KK       # 20736
W2R = KFLAT // NCORES  # 2592
NB = 288
WSC = 16.0           # fp8 weight scale
GSC = 256.0          # fp8 gate scale

TAPS = [(dz, dy, dx) for dz in (-1, 0, 1) for dy in (-1, 0, 1) for dx in (-1, 0, 1)]


def _tidx(dz, dy, dx):
    return (dz + 1) * 9 + (dy + 1) * 3 + (dx + 1)


# DoubleRow tap pairs: the pair stride (byte delta between the two moving
# windows) must be EVEN, so pair dx=-1 with dx=+1 (delta 2), and the dx=0
# column across dz/dy (deltas 676 / 26).
PAIRS = ([(_tidx(dz, dy, -1), _tidx(dz, dy, 1))
          for dz in (-1, 0, 1) for dy in (-1, 0, 1)]
         + [(_tidx(-1, dy, 0), _tidx(0, dy, 0)) for dy in (-1, 0, 1)]
         + [(_tidx(1, -1, 0), _tidx(1, 0, 0)), (_tidx(1, 1, 0), None)])
NPR = len(PAIRS)     # 14

_CACHE = {}

Copy = mybir.ActivationFunctionType.Copy
Iden = mybir.ActivationFunctionType.Identity
Gelu = mybir.ActivationFunctionType.Gelu
Sigmoid = mybir.ActivationFunctionType.Sigmoid
Square = mybir.ActivationFunctionType.Square
Sqrt = mybir.ActivationFunctionType.Sqrt
Relu = mybir.ActivationFunctionType.Relu
ADD = mybir.AluOpType.add
SUB = mybir.AluOpType.subtract
MULT = mybir.AluOpType.mult
DR = mybir.MatmulPerfMode.DoubleRow

# smalls columns
(S_Y, S_BA, S_BB, S_BC, S_MODB, S_OPB, S_N2W, S_N2B, S_N3W, S_N3B,
 S_GNG, S_GNB, S_N2WL, S_N2BL, S_N2WH, S_N2BH) = range(16)


def _toff(dz, dy, dx):
    return dz * PPL + dy * 26 + dx


def build_program():
    nc = bacc.Bacc("TRN2", target_bir_lowering=False)

    def dram_in(name, shape, dtype=F32):
        return nc.declare_dram_parameter(name, list(shape), dtype, isOutput=False)

    x_halo = dram_in("x_halo", [C, Z7 * PL], BF16)
    smalls = dram_in("smalls", [C, 16])
    knb1 = dram_in("knb1", [HID])
    knb2r = dram_in("knb2r", [KFLAT])
    gind6 = dram_in("gind6", [CT, 128, G])
    gexpT = dram_in("gexpT", [G, C])
    ident = dram_in("ident", [128, 128], BF16)
    onesc = dram_in("onesc", [128, 1], BF16)
    wAT = dram_in("wAT", [C, C], BF16)
    wBT = dram_in("wBT", [C, C], BF16)
    wCT = dram_in("wCT", [C, C], BF16)
    modWT = dram_in("modWT", [2 * C, C], BF16)
    w1T = dram_in("w1T", [2 * C, HID], BF16)
    w2m = dram_in("w2m", [HT, 128, W2R], BF16)
    opT = dram_in("opT", [C, C], BF16)
    wi8 = dram_in("wi8", [HT, 128, 3 * 256], FP8)
    d8 = dram_in("d8", [HT, 128, NPR * 256], FP8)
    wo8 = dram_in("wo8", [CT, 128, 6 * 256], FP8)
    out = nc.declare_dram_parameter("out", [C, VC], F32, isOutput=True)
    import os as _os
    DBG = _os.environ.get("BLK3D_DBG") == "1"
    if DBG:
        dbg_mod = nc.declare_dram_parameter("dbg_mod", [C, 1], F32, isOutput=True)
        dbg_kern = nc.declare_dram_parameter("dbg_kern", [C, KK], F32, isOutput=True)
        dbg_dyn = nc.declare_dram_parameter("dbg_dyn", [C, W5], F32, isOutput=True)
        dbg_xb = nc.declare_dram_parameter("dbg_xb", [C, W5], F32, isOutput=True)
        dbg_xln = nc.declare_dram_parameter("dbg_xln", [3 * 128, 2 * W5], F32, isOutput=True)
        dbg_gate = nc.declare_dram_parameter("dbg_gate", [6 * 128, 2 * VC], F32, isOutput=True)
        dbg_y = nc.declare_dram_parameter("dbg_y", [C, VC], F32, isOutput=True)
        dbg_comb = nc.declare_dram_parameter("dbg_comb", [C, 2], F32, isOutput=True)
        dbg_kp1 = nc.declare_dram_parameter("dbg_kp1", [HID, 1], F32, isOutput=True)
        dbg_krow = nc.declare_dram_parameter("dbg_krow", [1, W2R], F32, isOutput=True)

    with tile.TileContext(nc) as tc, ExitStack() as ctx:
        dram = ctx.enter_context(tc.tile_pool(name="dram", bufs=1, space="DRAM"))
        persist = ctx.enter_context(tc.tile_pool(name="persist", bufs=1))
        gpool = ctx.enter_context(tc.tile_pool(name="gemv", bufs=2))

        # ---------------- persistent small tiles ----------------
        sm = [persist.tile([128, 16], F32, name=f"sm{i}", tag=f"sm{i}")
              for i in range(CT)]
        for i in range(CT):
            nc.sync.dma_start(sm[i][:], smalls[128 * i:128 * (i + 1), :])
        id_t = persist.tile([128, 128], BF16, name="identt", tag="identt")
        nc.sync.dma_start(id_t[:], ident[:, :])
        ones_t = persist.tile([128, 1], BF16, name="onest", tag="onest")
        nc.sync.dma_start(ones_t[:], onesc[:, :])
        eps_t = persist.tile([128, 1], F32, name="epst", tag="epst")
        nc.vector.memset(eps_t[:], EPS)
        junk = persist.tile([128, VC], BF16, name="junk", tag="junk")
        kern = [persist.tile([128, KK], F32, name=f"kern{i}", tag=f"kern{i}")
                for i in range(CT)]
        cb_cols = [persist.tile([128, 1], F32, name=f"cbc{m}", tag=f"cbc{m}")
                   for m in range(CT)]

        xbp = ctx.enter_context(tc.tile_pool(name="xbp", bufs=1))
        xb = [xbp.tile([128, W5], BF16, name=f"xb{i}", tag=f"xb{i}")
              for i in range(CT)]

        def ln_rows(pool, tiles, width, tag):
            """per-voxel mean/rstd over C -> bf16 bcast tiles [128, width]"""
            with (tc.tile_pool(name=f"{tag}ps", bufs=2, space="PSUM") as lps,
                  tc.tile_pool(name=f"{tag}sq", bufs=2) as sqp,
                  tc.tile_pool(name=f"{tag}rw", bufs=1) as rwp):
                nch = (width + 511) // 512
                row = rwp.tile([1, 2 * width], F32, name="row", tag="row")
                for cidx in range(nch):
                    o0 = 512 * cidx
                    n = min(512, width - o0)
                    ps1 = lps.tile([1, 512], F32, name="s1", tag="s1")
                    for k in range(CT):
                        nc.tensor.matmul(ps1[:, 0:n], ones_t[:],
                                         tiles[k][:, o0:o0 + n],
                                         start=(k == 0), stop=(k == CT - 1))
                    nc.scalar.activation(row[:, o0:o0 + n], ps1[:, 0:n], Copy,
                                         scale=1.0 / C)
                for cidx in range(nch):
                    o0 = 512 * cidx
                    n = min(512, width - o0)
                    ps2 = lps.tile([1, 512], F32, name="s2", tag="s2")
                    for k in range(CT):
                        sq = sqp.tile([128, 512], BF16, name="sq", tag="sq")
                        nc.vector.tensor_mul(sq[:, 0:n], tiles[k][:, o0:o0 + n],
                                             tiles[k][:, o0:o0 + n])
                        nc.tensor.matmul(ps2[:, 0:n], ones_t[:], sq[:, 0:n],
                                         start=(k == 0), stop=(k == CT - 1))
                    nc.scalar.activation(row[:, width + o0:width + o0 + n],
                                         ps2[:, 0:n], Copy, scale=1.0 / C)
                # spread each half to [96, w] for cheap elementwise math
                w96 = width // 96
                rs = rwp.tile([96, 2 * w96], F32, name="rs", tag="rs")
                nc.gpsimd.dma_start(rs[:, 0:w96], row[:, 0:width])
                nc.gpsimd.dma_start(rs[:, w96:2 * w96], row[:, width:2 * width])
                m2 = rwp.tile([96, w96], F32, name="m2", tag="m2")
                nc.scalar.square(m2[:], rs[:, 0:w96])
                vr = rwp.tile([96, w96], F32, name="vr", tag="vr")
                nc.vector.tensor_sub(vr[:], rs[:, w96:2 * w96], m2[:])
                nc.scalar.activation(vr[:], vr[:], Sqrt, bias=eps_t[0:96, 0:1])
                nc.vector.reciprocal(vr[:], vr[:])
                mrow = rwp.tile([1, 2 * width], BF16, name="mrow", tag="mrow")
                nc.gpsimd.dma_start(mrow[:, 0:width], rs[:, 0:w96])
                nc.gpsimd.dma_start(mrow[:, width:2 * width], vr[:])
                muB = pool.tile([128, width], BF16, name=f"{tag}mu",
                                tag=f"{tag}mu")
                rsB = pool.tile([128, width], BF16, name=f"{tag}rs",
                                tag=f"{tag}rs")
                nc.gpsimd.partition_broadcast(muB[:], mrow[0:1, 0:width])
                nc.gpsimd.partition_broadcast(rsB[:], mrow[0:1, width:2 * width])
            return muB, rsB

        # =================== phases A-C: scoped (xs/mv/dyn die after) ======
        with (tc.tile_pool(name="xsp", bufs=1) as xsp,
              tc.tile_pool(name="dynp", bufs=1) as dynp,
              tc.tile_pool(name="opTp", bufs=1) as opTp):
            xs = [xsp.tile([128, Z7 * PL], BF16, name=f"xs{i}", tag=f"xs{i}")
                  for i in range(CT)]
            for i in range(CT):
                nc.sync.dma_start(xs[i][:], x_halo[128 * i:128 * (i + 1), :])
            dyn = [dynp.tile([128, W5], BF16, name=f"dyn{i}", tag=f"dyn{i}")
                   for i in range(CT)]
            opT_t = [opTp.tile([128, C], BF16, name=f"opT{i}", tag=f"opT{i}")
                     for i in range(CT)]
            for i in range(CT):
                nc.sync.dma_start(opT_t[i][:], opT[128 * i:128 * (i + 1), :])

            # ---- vc partials + AG1 ----
            vcs = persist.tile([128, CT], F32, name="vcs", tag="vcs")
            for i in range(CT):
                nc.scalar.activation(junk[:], xs[i][:, 2 * PL:2 * PL + VC],
                                     Copy, accum_out=vcs[:, i:i + 1])
            ag1i = dram.tile([C], F32, name="ag1i", tag="ag1i")
            ag1o = dram.tile([NCORES, C], F32, name="ag1o", tag="ag1o",
                             addr_space="Shared")
            nc.gpsimd.dma_start(
                bass.AP(tensor=ag1i[:].tensor, offset=ag1i[:].offset,
                        ap=[[1, 128], [128, CT]]), vcs[:])
            nc.gpsimd.collective_compute(
                "AllGather", mybir.AluOpType.bypass,
                replica_groups=[list(range(NCORES))], ins=[ag1i[:]],
                outs=[ag1o[:]])

            # ---- text chain / mod / kp1 (row-form gemv) ----
            _psRs = ExitStack()
            psR = _psRs.enter_context(
                tc.tile_pool(name="psR", bufs=1, space="PSUM"))
            pcs = [psR.tile([1, 512], F32, name=f"rp{j}", tag=f"rowps{j}")
                   for j in range(6)]
            with tc.tile_pool(name="wstream", bufs=2) as wpool:

                def gemv_chain(wdram, in_cols, nk, nm, act, bias_col, tag,
                               odt=BF16, bias_t=None):
                    # out = W @ in  via moving-weights row matmuls; the row
                    # [1, 128*nm] is transposed to columns through DRAM.
                    width = 128 * nm
                    npc = (width + 511) // 512
                    for k in range(nk):
                        wt = wpool.tile([128, width], BF16, name=f"{tag}w",
                                        tag=f"{tag}w")
                        nc.sync.dma_start(wt[:],
                                          wdram[128 * k:128 * (k + 1), :])
                        for pc in range(npc):
                            o0 = 512 * pc
                            n = min(512, width - o0)
                            nc.tensor.matmul(pcs[pc][:, 0:n], in_cols[k][:],
                                             wt[:, o0:o0 + n],
                                             start=(k == 0),
                                             stop=(k == nk - 1))
                    row = gpool.tile([1, width], BF16, name=f"{tag}row",
                                     tag=f"{tag}row")
                    for pc in range(npc):
                        o0 = 512 * pc
                        n = min(512, width - o0)
                        nc.scalar.activation(row[:, o0:o0 + n],
                                             pcs[pc][:, 0:n], Copy)
                    drow = dram.tile([width], BF16, name=f"{tag}dr",
                                     tag=f"{tag}dr")
                    nc.gpsimd.dma_start(drow[:], row[:])
                    ct = gpool.tile([128, nm], BF16, name=f"{tag}ct",
                                    tag=f"{tag}ct")
                    nc.gpsimd.dma_start(
                        ct[:], bass.AP(tensor=drow[:].tensor,
                                       offset=drow[:].offset,
                                       ap=[[1, 128], [128, nm]]))
                    outs = []
                    for m in range(nm):
                        if bias_t is not None:
                            bias = bias_t[:, m:m + 1]
                        else:
                            bias = sm[m][:, bias_col:bias_col + 1]
                        o = gpool.tile([128, 1], odt, name=f"{tag}o{m}",
                                       tag=f"{tag}o{m}")
                        nc.scalar.activation(o[:], ct[:, m:m + 1], act,
                                             bias=bias)
                        outs.append(o)
                    return outs

                y_cols = []
                for i in range(CT):
                    t = gpool.tile([128, 1], BF16, name=f"yc{i}", tag=f"yc{i}")
                    nc.scalar.activation(t[:], sm[i][:, S_Y:S_Y + 1], Copy)
                    y_cols.append(t)
                hA = gemv_chain(wAT, y_cols, CT, CT, Relu, S_BA, "wa")
                hB = gemv_chain(wBT, hA, CT, CT, Iden, S_BB, "wb")
                attn = gemv_chain(wCT, hB, CT, CT, Iden, S_BC, "wc")

                vc8 = persist.tile([128, CT * NCORES], F32, name="vc8",
                                   tag="vc8")
                nc.gpsimd.dma_start(
                    bass.AP(tensor=vc8[:].tensor, offset=vc8[:].offset,
                            ap=[vc8[:].ap[0], [CT, NCORES], [1, CT]]),
                    bass.AP(tensor=ag1o[:].tensor, offset=ag1o[:].offset,
                            ap=[[1, 128], [C, NCORES], [128, CT]]))
                v3 = vc8.rearrange("p (i k) -> p i k", i=NCORES, k=CT)
                for step in (4, 2, 1):
                    nc.vector.tensor_add(v3[:, 0:step, :], v3[:, 0:step, :],
                                         v3[:, step:2 * step, :])
                comb = []
                for i in range(CT):
                    cb = gpool.tile([128, 1], BF16, name=f"cmb{i}",
                                    tag=f"cmb{i}")
                    nc.scalar.activation(cb[:], vc8[:, i:i + 1], Copy,
                                         scale=1.0 / V)
                    comb.append(cb)
                comb += attn

                mod = gemv_chain(modWT, comb, 2 * CT, CT, Sigmoid, S_MODB,
                                 "md", odt=F32)

                knb1_t = persist.tile([128, HT], F32, name="knb1t",
                                      tag="knb1t")
                nc.gpsimd.dma_start(
                    knb1_t[:],
                    bass.AP(tensor=knb1, offset=0, ap=[[1, 128], [128, HT]]))
                kp1 = gemv_chain(w1T, comb, 2 * CT, HT, Relu, None, "k1",
                                 bias_t=knb1_t)

            if DBG:
                with tc.tile_pool(name="dbg0p", bufs=2) as dbg0p:
                    for i in range(CT):
                        t = dbg0p.tile([128, 2], F32, name="dbgc", tag="dbgc")
                        nc.scalar.activation(t[:, 0:1], comb[i][:], Copy)
                        nc.scalar.activation(t[:, 1:2], comb[CT + i][:], Copy)
                        nc.gpsimd.dma_start(dbg_comb[128 * i:128 * (i + 1), :],
                                            t[:])
                    for m in range(HT):
                        t = dbg0p.tile([128, 1], F32, name="dbgk1", tag="dbgk1")
                        nc.scalar.activation(t[:], kp1[m][:], Copy)
                        nc.gpsimd.dma_start(
                            dbg_kp1[128 * m:128 * (m + 1), :], t[:])

            # ---- W2 own rows + AG2 -> kernels ----
            ag2i = dram.tile([W2R], F32, name="ag2i", tag="ag2i")
            ag2o = dram.tile([NCORES, W2R], F32, name="ag2o", tag="ag2o",
                             addr_space="Shared")
            PIECES = [(0, 512), (512, 512), (1024, 512), (1536, 512),
                      (2048, 512), (2560, 32)]
            with (tc.tile_pool(name="w2s", bufs=3) as w2s,
                  tc.tile_pool(name="krowp", bufs=1) as krowp):
                pcs = [psR.tile([1, 512], F32, name=f"w2p{j}",
                                tag=f"rowps{j}") for j in range(6)]
                for k in range(HT):
                    wt = w2s.tile([128, W2R], BF16, name="w2w", tag="w2w")
                    nc.sync.dma_start(wt[:], w2m[k, :, :])
                    for j, (o0, n) in enumerate(PIECES):
                        nc.tensor.matmul(pcs[j][:, 0:n], kp1[k][:],
                                         wt[:, o0:o0 + n], start=(k == 0),
                                         stop=(k == HT - 1))
                krow = krowp.tile([1, W2R], F32, name="krow", tag="krow")
                for j, (o0, n) in enumerate(PIECES):
                    nc.scalar.activation(krow[:, o0:o0 + n], pcs[j][:, 0:n],
                                         Copy)
                nc.gpsimd.dma_start(ag2i[:], krow[:])
                if DBG:
                    nc.gpsimd.dma_start(dbg_krow[:, :], krow[:])
            nc.gpsimd.collective_compute(
                "AllGather", mybir.AluOpType.bypass,
                replica_groups=[list(range(NCORES))], ins=[ag2i[:]],
                outs=[ag2o[:]])
            for i in range(CT):
                nc.gpsimd.dma_start(
                    kern[i][:],
                    bass.AP(tensor=ag2o[:].tensor,
                            offset=ag2o[:].offset + 128 * i * KK,
                            ap=[[KK, 128], [1, KK]]))
                kb = gpool.tile([128, KK], F32, name="kernb", tag="kernb")
                nc.gpsimd.dma_start(
                    kb[:], bass.AP(tensor=knb2r, offset=128 * i * KK,
                                   ap=[[KK, 128], [1, KK]]))
                nc.vector.tensor_add(kern[i][:], kern[i][:], kb[:])
            _psRs.close()

            # ---- mv staging + dyn conv + GN stats ----
            ag3i = dram.tile([G, 2], F32, name="ag3i", tag="ag3i")
            ag3o = dram.tile([NCORES, G, 2], F32, name="ag3o", tag="ag3o",
                             addr_space="Shared")
            with (tc.tile_pool(name="mvp", bufs=2) as mvp,
                  tc.tile_pool(name="diagp", bufs=2) as diagp,
                  tc.tile_pool(name="cvps", bufs=6, space="PSUM") as cvps,
                  tc.tile_pool(name="gnps", bufs=1, space="PSUM") as gnps,
                  tc.tile_pool(name="gnst", bufs=2) as gnst):
                gps = gnps.tile([G, 2], F32, name="gps", tag="gps")
                for i in range(CT):
                    mv = mvp.tile([128, MVN], BF16, name="mv", tag="mv")
                    if i < 2:
                        nc.gpsimd.memset(mv[:], 0.0)
                    for z in range(Z7):
                        for yh in range(2):
                            dst = bass.AP(
                                tensor=mv[:].tensor,
                                offset=(mv[:].offset + HDR + z * PPL
                                        + yh * 312),
                                ap=[mv[:].ap[0], [26, 12], [1, 24]])
                            nc.vector.tensor_scalar_mul(
                                dst,
                                xs[i][:, z * PL + yh * NB:
                                      z * PL + yh * NB + NB],
                                mod[i][:, 0:1])
                    dg = [diagp.tile([128, 128], BF16, name=f"dg{t}",
                                     tag=f"dg{t}") for t in range(KK)]
                    for t in range(KK):
                        nc.vector.tensor_scalar_mul(dg[t][:], id_t[:],
                                                    kern[i][:, t:t + 1])
                    for o in range(Z5):
                        for bh in range(2):
                            ps = cvps.tile([128, NB], F32, name="cv",
                                           tag="cv")
                            base = (mv[:].offset + HDR + o * PPL + bh * 312)
                            for t, (dz, dy, dx) in enumerate(TAPS):
                                sv = bass.AP(
                                    tensor=mv[:].tensor,
                                    offset=base + _toff(1 + dz, dy, dx),
                                    ap=[mv[:].ap[0], [26, 12], [1, 24]])
                                nc.tensor.matmul(ps[:], dg[t][:], sv,
                                                 start=(t == 0),
                                                 stop=(t == KK - 1))
                            nc.scalar.activation(
                                dyn[i][:, o * PL + bh * NB:
                                       o * PL + bh * NB + NB],
                                ps[:], Copy)
                    st = gnst.tile([128, 2], F32, name="gnstat", tag="gnstat")
                    nc.scalar.activation(junk[:], dyn[i][:, PL:PL + VC], Copy,
                                         accum_out=st[:, 0:1])
                    nc.scalar.activation(junk[:], dyn[i][:, PL:PL + VC],
                                         Square, accum_out=st[:, 1:2])
                    gi = gnst.tile([128, G], F32, name="gind", tag="gind")
                    nc.gpsimd.dma_start(gi[:], gind6[i, :, :])
                    nc.tensor.matmul(gps[:], gi[:], st[:], start=(i == 0),
                                     stop=(i == CT - 1))
                gsb = persist.tile([G, 2], F32, name="gsb", tag="gsb")
                nc.scalar.activation(gsb[:], gps[:], Copy)
            nc.gpsimd.dma_start(ag3i[:], gsb[:])
            nc.gpsimd.collective_compute(
                "AllGather", mybir.AluOpType.bypass,
                replica_groups=[list(range(NCORES))], ins=[ag3i[:]],
                outs=[ag3o[:]])

            gst8 = persist.tile([G, 2 * NCORES], F32, name="gst8", tag="gst8")
            nc.gpsimd.dma_start(
                bass.AP(tensor=gst8[:].tensor, offset=gst8[:].offset,
                        ap=[gst8[:].ap[0], [2, NCORES], [1, 2]]),
                bass.AP(tensor=ag3o[:].tensor, offset=ag3o[:].offset,
                        ap=[[2, G], [2 * G, NCORES], [1, 2]]))
            g3 = gst8.rearrange("p (i t) -> p i t", i=NCORES, t=2)
            for step in (4, 2, 1):
                nc.vector.tensor_add(g3[:, 0:step, :], g3[:, 0:step, :],
                                     g3[:, step:2 * step, :])
            NGRP = float(GD * V)
            gmr = persist.tile([G, 2], F32, name="gmr", tag="gmr")
            nc.scalar.activation(gmr[:, 0:1], gst8[:, 0:1], Copy,
                                 scale=1.0 / NGRP)
            musq = persist.tile([G, 1], F32, name="musq", tag="musq")
            nc.scalar.square(musq[:], gmr[:, 0:1])
            var = persist.tile([G, 1], F32, name="gvar", tag="gvar")
            nc.vector.tensor_scalar(var[:], gst8[:, 1:2], 1.0 / NGRP, None,
                                    op0=MULT)
            nc.vector.tensor_sub(var[:], var[:], musq[:])
            nc.scalar.activation(var[:], var[:], Sqrt, bias=eps_t[0:G, 0:1])
            nc.vector.reciprocal(gmr[:, 1:2], var[:])

            with (tc.tile_pool(name="gnf", bufs=2) as gnf,
                  tc.tile_pool(name="psA", bufs=2, space="PSUM") as psA):
                shifts = []
                gscs = []
                for i in range(CT):
                    ge = gnf.tile([G, 128], F32, name=f"gexp{i}",
                                  tag=f"gexp{i}")
                    nc.gpsimd.dma_start(ge[:], gexpT[:, 128 * i:128 * (i + 1)])
                    ps = psA.tile([128, 2], F32, name="gn2", tag="gvps")
                    nc.tensor.matmul(ps[:], ge[:], gmr[:], start=True,
                                     stop=True)
                    mu_c = gnf.tile([128, 2], F32, name=f"muc{i}",
                                    tag=f"muc{i}")
                    nc.scalar.activation(mu_c[:], ps[:], Copy)
                    a = persist.tile([128, 1], F32, name=f"gsc{i}",
                                     tag=f"gsc{i}")
                    nc.vector.tensor_mul(a[:], sm[i][:, S_GNG:S_GNG + 1],
                                         mu_c[:, 1:2])
                    b = gnf.tile([128, 1], BF16, name=f"gsh{i}", tag=f"gsh{i}")
                    t = gnf.tile([128, 1], F32, name="gtmp", tag="gtmp")
                    nc.vector.tensor_mul(t[:], mu_c[:, 0:1], a[:])
                    nc.vector.tensor_sub(t[:], sm[i][:, S_GNB:S_GNB + 1], t[:])
                    nc.scalar.activation(b[:], t[:], Copy)
                    shifts.append(b)
                    gscs.append(a)
                for m in range(CT):
                    ps = psA.tile([128, 1], F32, name="cbp", tag="gvps")
                    for k in range(CT):
                        nc.tensor.matmul(ps[:],
                                         opT_t[k][:, 128 * m:128 * (m + 1)],
                                         shifts[k][:], start=(k == 0),
                                         stop=(k == CT - 1))
                    nc.scalar.activation(cb_cols[m][:], ps[:], Iden,
                                         bias=sm[m][:, S_OPB:S_OPB + 1])
                for i in range(CT):
                    nc.vector.tensor_scalar_mul(opT_t[i][:], opT_t[i][:],
                                                gscs[i][:])

            # cafm matmul (5 planes) + xb = (psum + cb) * x
            CH5 = [(0, 512), (512, 512), (1024, 512), (1536, 512),
                   (2048, 512), (2560, 320)]
            with tc.tile_pool(name="opwps", bufs=2, space="PSUM") as opwps:
                for m in range(CT):
                    for o0, n in CH5:
                        ps = opwps.tile([128, 512], F32, name="opw", tag="opw")
                        for k in range(CT):
                            nc.tensor.matmul(
                                ps[:, 0:n],
                                opT_t[k][:, 128 * m:128 * (m + 1)],
                                dyn[k][:, o0:o0 + n], start=(k == 0),
                                stop=(k == CT - 1))
                        nc.vector.scalar_tensor_tensor(
                            xb[m][:, o0:o0 + n], ps[:, 0:n], cb_cols[m][:],
                            xs[m][:, PL + o0:PL + o0 + n], op0=ADD, op1=MULT)

            if DBG:
                with tc.tile_pool(name="dbgp", bufs=2) as dbgp:
                    for i in range(CT):
                        t = dbgp.tile([128, W5], F32, name="dbgt", tag="dbgt")
                        nc.scalar.activation(t[:], dyn[i][:], Copy)
                        nc.gpsimd.dma_start(dbg_dyn[128 * i:128 * (i + 1), :],
                                            t[:])
                        t2 = dbgp.tile([128, W5], F32, name="dbgt2",
                                       tag="dbgt2")
                        nc.scalar.activation(t2[:], xb[i][:], Copy)
                        nc.gpsimd.dma_start(dbg_xb[128 * i:128 * (i + 1), :],
                                            t2[:])
                        nc.gpsimd.dma_start(dbg_kern[128 * i:128 * (i + 1), :],
                                            kern[i][:])
                        t3 = dbgp.tile([128, 1], F32, name="dbgt3",
                                       tag="dbgt3")
                        nc.scalar.activation(t3[:], mod[i][:], Copy)
                        nc.gpsimd.dma_start(dbg_mod[128 * i:128 * (i + 1), :],
                                            t3[:])

        # =================== LN2 -> xln8 (fp8, channel-paired) =============
        xln8p = ctx.enter_context(tc.tile_pool(name="xln8p", bufs=1))
        xln8 = [xln8p.tile([128, 2 * W5], FP8, name=f"xl{q}", tag=f"xl{q}")
                for q in range(3)]
        with tc.tile_pool(name="lnbp", bufs=1) as lnbp:
            muB, rsB = ln_rows(lnbp, xb, W5, "ln2")
            with tc.tile_pool(name="lnt", bufs=2) as lnt:
                for i in range(CT):
                    t1 = lnt.tile([128, W5], BF16, name="lnt1", tag="lnt1")
                    nc.vector.tensor_sub(t1[:], xb[i][:], muB[:])
                    nc.vector.tensor_mul(t1[:], t1[:], rsB[:])
                    q, j = i // 2, i % 2
                    dst = xln8[q][:, j * W5:(j + 1) * W5]
                    nc.scalar.activation(dst[:, 0:PL], t1[:, 0:PL], Iden,
                                         bias=sm[i][:, S_N2BL:S_N2BL + 1],
                                         scale=sm[i][:, S_N2WL:S_N2WL + 1])
                    nc.scalar.activation(dst[:, PL:4 * PL], t1[:, PL:4 * PL],
                                         Iden,
                                         bias=sm[i][:, S_N2B:S_N2B + 1],
                                         scale=sm[i][:, S_N2W:S_N2W + 1])
                    nc.scalar.activation(dst[:, 4 * PL:5 * PL],
                                         t1[:, 4 * PL:5 * PL], Iden,
                                         bias=sm[i][:, S_N2BH:S_N2BH + 1],
                                         scale=sm[i][:, S_N2WH:S_N2WH + 1])

        # =================== MLP: fp8 DoubleRow ===========================
        gate8p = ctx.enter_context(tc.tile_pool(name="gate8p", bufs=1))
        gate8 = [gate8p.tile([128, 2 * VC], FP8, name=f"g8{q}", tag=f"g8{q}")
                 for q in range(6)]
        ytp = ctx.enter_context(tc.tile_pool(name="ytp", bufs=1))
        y_t = [ytp.tile([128, VC], BF16, name=f"y{i}", tag=f"y{i}")
               for i in range(CT)]

        pair_off = []
        for t0, t1 in PAIRS:
            o0 = _toff(1 + TAPS[t0][0], TAPS[t0][1], TAPS[t0][2])
            if t1 is not None:
                d = _toff(1 + TAPS[t1][0], TAPS[t1][1], TAPS[t1][2]) - o0
            else:
                d = 2  # dead pair slot (zero weights), even stride
            pair_off.append((o0, d))
        assert all(d > 0 and d % 2 == 0 for _, d in pair_off)

        with (tc.tile_pool(name="hpadp", bufs=3) as hpad_pool,
              tc.tile_pool(name="wi8p", bufs=3) as wi8p,
              tc.tile_pool(name="d8p", bufs=3) as d8p,
              tc.tile_pool(name="glueG", bufs=2) as glueG,
              tc.tile_pool(name="wips", bufs=3, space="PSUM") as wips,
              tc.tile_pool(name="cvp2", bufs=5, space="PSUM") as cvp2):

            nmlp = [0]

            def mlp_tile(tt, sink):
                """Wi (fp8 DR) -> staged padded h8 -> conv (fp8 DR pairs);
                sink(nb, ps) consumes each conv psum block immediately."""
                wt = wi8p.tile([128, 3 * 256], FP8, name="wi8t", tag="wi8t")
                nc.sync.dma_start(wt[:], wi8[tt, :, :])
                w4 = wt.rearrange("p (q j m) -> p q j m", q=3, j=2, m=128)
                hp = hpad_pool.tile([128, HPN], FP8, name="hpad", tag="hpad")
                if nmlp[0] < 3:
                    nc.gpsimd.memset(hp[:], 0.0)
                nmlp[0] += 1
                for z in range(Z5):
                    for yh in range(2):
                        ps = wips.tile([128, NB], F32, name="wi_ps",
                                       tag="wi_ps")
                        for q in range(3):
                            mv_ = bass.AP(
                                tensor=xln8[q][:].tensor,
                                offset=(xln8[q][:].offset + z * PL + yh * NB),
                                ap=[xln8[q][:].ap[0], [W5, 2], [1, NB]])
                            nc.tensor.matmul(ps[:], w4[:, q], mv_,
                                             start=(q == 0), stop=(q == 2),
                                             perf_mode=DR)
                        dst = bass.AP(
                            tensor=hp[:].tensor,
                            offset=hp[:].offset + HDR + z * PPL + yh * 312,
                            ap=[hp[:].ap[0], [26, 12], [1, 24]])
                        if z in (1, 3):
                            nc.vector.tensor_scalar_mul(dst, ps[:], 1.0 / WSC)
                        else:
                            nc.scalar.activation(dst, ps[:], Copy,
                                                 scale=1.0 / WSC)
                dgt = d8p.tile([128, NPR * 256], FP8, name="d8t", tag="d8t")
                nc.sync.dma_start(dgt[:], d8[tt, :, :])
                dg4 = dgt.rearrange("p (r j m) -> p r j m", r=NPR, j=2, m=128)
                for o in range(ZP):
                    for bh in range(2):
                        # stream full padded rows (312 wide) so the moving AP
                        # stays 3-dim; consumer reads interior cells strided
                        ps = cvp2.tile([128, 312], F32, name="cv2", tag="cv2")
                        base = hp[:].offset + HDR + (o + 1) * PPL + bh * 312
                        for pr in range(NPR):
                            o0, dlt = pair_off[pr]
                            sv = bass.AP(
                                tensor=hp[:].tensor, offset=base + o0 - PPL,
                                ap=[hp[:].ap[0], [dlt, 2], [1, 312]])
                            nc.tensor.matmul(ps[:], dg4[:, pr], sv,
                                             start=(pr == 0),
                                             stop=(pr == NPR - 1),
                                             perf_mode=DR)
                        sink(o * 2 + bh, ps)

            for u in range(HH):
                def _interior(ps):
                    return bass.AP(tensor=ps[:].tensor, offset=ps[:].offset,
                                   ap=[ps[:].ap[0], [26, 12], [1, 24]])

                g1 = glueG.tile([128, VC], BF16, name="gelu1", tag="gelu1")
                mlp_tile(u, lambda nb, ps: nc.scalar.activation(
                    g1[:, NB * nb:NB * (nb + 1)], _interior(ps), Gelu,
                    scale=1.0 / WSC))
                q, j = u // 2, u % 2
                # gate = (gelu(c1) * GSC/WSC) * conv2, reading conv2 psum
                # blocks directly (scale folded: GSC/WSC applied to g1)
                mlp_tile(u + HH, lambda nb, ps: nc.vector.scalar_tensor_tensor(
                    gate8[q][:, j * VC + NB * nb:j * VC + NB * (nb + 1)],
                    g1[:, NB * nb:NB * (nb + 1)], GSC / WSC, _interior(ps),
                    op0=MULT, op1=MULT))

        with (tc.tile_pool(name="wo8p", bufs=2) as wo8p,
              tc.tile_pool(name="wops", bufs=2, space="PSUM") as wops):
            CH3 = [(0, 512), (512, 512), (1024, 512), (1536, 192)]
            for m in range(CT):
                wt = wo8p.tile([128, 6 * 256], FP8, name="wo8t", tag="wo8t")
                nc.sync.dma_start(wt[:], wo8[m, :, :])
                w4 = wt.rearrange("p (q j m) -> p q j m", q=6, j=2, m=128)
                for o0, n in CH3:
                    ps = wops.tile([128, 512], F32, name="wo_ps", tag="wo_ps")
                    for q in range(6):
                        mv_ = bass.AP(
                            tensor=gate8[q][:].tensor,
                            offset=gate8[q][:].offset + o0,
                            ap=[gate8[q][:].ap[0], [VC, 2], [1, n]])
                        nc.tensor.matmul(ps[:, 0:n], w4[:, q], mv_,
                                         start=(q == 0), stop=(q == 5),
                                         perf_mode=DR)
                    nc.vector.scalar_tensor_tensor(
                        y_t[m][:, o0:o0 + n], ps[:, 0:n],
                        1.0 / (WSC * GSC), xb[m][:, PL + o0:PL + o0 + n],
                        op0=MULT, op1=ADD)

        if DBG:
            with tc.tile_pool(name="dbg2p", bufs=1) as dbg2p:
                for q in range(3):
                    t = dbg2p.tile([128, 2 * W5], F32, name="dbgx", tag="dbgx")
                    nc.scalar.activation(t[:], xln8[q][:], Copy)
                    nc.gpsimd.dma_start(dbg_xln[128 * q:128 * (q + 1), :], t[:])
                for q in range(6):
                    t = dbg2p.tile([128, 2 * VC], F32, name="dbgg", tag="dbgg")
                    nc.scalar.activation(t[:], gate8[q][:], Copy)
                    nc.gpsimd.dma_start(dbg_gate[128 * q:128 * (q + 1), :], t[:])
                for i in range(CT):
                    t = dbg2p.tile([128, VC], F32, name="dbgy", tag="dbgy")
                    nc.scalar.activation(t[:], y_t[i][:], Copy)
                    nc.gpsimd.dma_start(dbg_y[128 * i:128 * (i + 1), :], t[:])

        # =================== LN3 + output ==================================
        with tc.tile_pool(name="ln3bp", bufs=1) as ln3bp:
            muB3, rsB3 = ln_rows(ln3bp, y_t, VC, "ln3")
            with tc.tile_pool(name="glueH", bufs=2) as glueH:
                for i in range(CT):
                    t1 = glueH.tile([128, VC], BF16, name="ln3t", tag="ln3t")
                    nc.vector.tensor_sub(t1[:], y_t[i][:], muB3[:])
                    nc.vector.tensor_mul(t1[:], t1[:], rsB3[:])
                    of = glueH.tile([128, VC], F32, name="outf", tag="outf")
                    nc.scalar.activation(of[:], t1[:], Iden,
                                         bias=sm[i][:, S_N3B:S_N3B + 1],
                                         scale=sm[i][:, S_N3W:S_N3W + 1])
                    nc.gpsimd.dma_start(out[128 * i:128 * (i + 1), :], of[:])

    nc.compile()
    return nc


def _prep(inputs):
    bf = lambda a: np.ascontiguousarray(a).astype(BF)
    f32 = lambda a: np.ascontiguousarray(a, dtype=np.float32)
    fp8 = lambda a: np.ascontiguousarray(a).astype(E4)
    x = f32(inputs["x"][0])
    xf = x.reshape(C, S, PL)

    W_B = f32(inputs["tp_W"]) @ f32(inputs["lora_W2"])
    b_B = f32(inputs["tp_W"]) @ f32(inputs["lora_b2"]) + f32(inputs["tp_b"])
    W_C = f32(inputs["attn_Wo"]) @ f32(inputs["attn_Wv"])
    b_C = f32(inputs["attn_Wo"]) @ f32(inputs["attn_bv"]) + f32(inputs["attn_bo"])

    gind6 = np.zeros((CT, 128, G), np.float32)
    for j in range(CT):
        for p in range(128):
            gind6[j, p, (128 * j + p) // GD] = 1.0
    gexpT = np.zeros((G, C), np.float32)
    for c in range(C):
        gexpT[c // GD, c] = 1.0

    kn_W2 = f32(inputs["kn_W2"])
    kn_W1 = f32(inputs["kn_W1"])
    mlp_Wi = f32(inputs["mlp_Wi"])
    mlp_dw = f32(inputs["mlp_dw"]).reshape(HID, KK)
    mlp_Wo = f32(inputs["mlp_Wo"])

    wi8a = np.zeros((HT, 128, 3 * 256), np.float32)
    WiT = mlp_Wi.T
    for h in range(HT):
        blk = WiT[:, 128 * h:128 * (h + 1)]
        for q in range(3):
            for j in range(2):
                wi8a[h, :, q * 256 + j * 128:q * 256 + j * 128 + 128] = \
                    WSC * blk[256 * q + 128 * j:256 * q + 128 * (j + 1), :]
    d8a = np.zeros((HT, 128, NPR * 256), np.float32)
    idx = np.arange(128)
    for h in range(HT):
        for pr, (t0, t1) in enumerate(PAIRS):
            for j, t in enumerate((t0, t1)):
                if t is None:
                    continue
                d8a[h, idx, pr * 256 + j * 128 + idx] = \
                    WSC * mlp_dw[128 * h:128 * (h + 1), t]
    wo8a = np.zeros((CT, 128, 6 * 256), np.float32)
    WoT = mlp_Wo.T
    for m in range(CT):
        blk = WoT[:, 128 * m:128 * (m + 1)]
        for q in range(6):
            for j in range(2):
                wo8a[m, :, q * 256 + j * 128:q * 256 + j * 128 + 128] = \
                    WSC * blk[256 * q + 128 * j:256 * q + 128 * (j + 1), :]

    com = dict(
        gind6=gind6, gexpT=gexpT,
        ident=bf(np.eye(128, dtype=np.float32)),
        onesc=bf(np.ones((128, 1), np.float32)),
        wAT=bf(f32(inputs["lora_W1"]).T),
        wBT=bf(W_B.T), wCT=bf(W_C.T),
        modWT=bf(f32(inputs["mod_W"]).T),
        w1T=bf(kn_W1.T),
        knb1=f32(inputs["kn_b1"]), knb2r=f32(inputs["kn_b2"]),
        opT=bf(f32(inputs["op_W"]).T),
        wi8=fp8(wi8a), d8=fp8(d8a), wo8=fp8(wo8a),
    )

    n2w = f32(inputs["n2_w"]); n2b = f32(inputs["n2_b"])
    in_maps = []
    for i in range(NCORES):
        z0 = ZP * i
        xh = np.zeros((C, Z7, PL), np.float32)
        lo, hi = max(z0 - 2, 0), min(z0 + ZP + 2, S)
        xh[:, lo - (z0 - 2):lo - (z0 - 2) + (hi - lo)] = xf[:, lo:hi]
        lomask = 1.0 if i > 0 else 0.0
        himask = 1.0 if i < NCORES - 1 else 0.0
        smalls = np.zeros((C, 16), np.float32)
        smalls[:, S_Y] = f32(inputs["y"][0, 0])
        smalls[:, S_BA] = f32(inputs["lora_b1"])
        smalls[:, S_BB] = b_B
        smalls[:, S_BC] = b_C
        smalls[:, S_MODB] = f32(inputs["mod_b"])
        smalls[:, S_OPB] = f32(inputs["op_b"])
        smalls[:, S_N2W] = n2w
        smalls[:, S_N2B] = n2b
        smalls[:, S_N3W] = f32(inputs["n3_w"])
        smalls[:, S_N3B] = f32(inputs["n3_b"])
        smalls[:, S_GNG] = f32(inputs["gn_g"])
        smalls[:, S_GNB] = f32(inputs["gn_b"])
        smalls[:, S_N2WL] = n2w * lomask
        smalls[:, S_N2BL] = n2b * lomask
        smalls[:, S_N2WH] = n2w * himask
        smalls[:, S_N2BH] = n2b * himask
        m = dict(com)
        m.update(
            x_halo=xh.reshape(C, Z7 * PL).astype(BF),
            smalls=smalls,
            w2m=bf(kn_W2[W2R * i:W2R * (i + 1), :].T.reshape(HT, 128, W2R)),
        )
        in_maps.append(m)
    return in_maps


def kernel(**inputs) -> np.ndarray:
    if "nc" not in _CACHE:
        _CACHE["nc"] = build_program()
    nc = _CACHE["nc"]
    in_maps = _prep(inputs)
    res = run_bass_kernel_spmd(nc, in_maps, list(range(NCORES)))
    outs = [res.results[i]["out"].reshape(C, ZP, PL) for i in range(NCORES)]
    full = np.concatenate(outs, axis=1)
    return full.reshape(1, C, S, S, S).astype(np.float32)


# revision 5
# speedup vs baseline: 1.0652x; 1.0112x over previous
"""Bass/Trainium2 SPMD kernel for nn_Block3D (8 NeuronCores) — v2.

z-shard (3 planes/core) with a 2-deep halo (7 input planes) so the LN2
halo exchange is computed locally instead of AllGathered. Collectives are
three small AllGathers (vc partial sums, kernel_net rows, GN stats) with
local reduction. The whole MLP (Wi / depthwise conv / Wo) runs in fp8
DoubleRow (tap-pair matmuls, 4x MAC rate); the CAFM dynamic conv stays
bf16 diag-matmul on PE with DVE-built diagonal stationaries.
"""

from contextlib import ExitStack

import numpy as np
import ml_dtypes

import concourse.bass as bass
import concourse.bacc as bacc
import concourse.tile as tile
from concourse import mybir
from concourse.bass_utils import run_bass_kernel_spmd

BF = ml_dtypes.bfloat16
E4 = ml_dtypes.float8_e4m3fn
F32 = mybir.dt.float32
BF16 = mybir.dt.bfloat16
FP8 = mybir.dt.float8e4

C = 768
G = 12
GD = 64
S = 24
HID = 4 * C          # 3072
HT = HID // 128      # 24
HH = HT // 2         # 12
CT = C // 128        # 6
KK = 27
V = S * S * S
EPS = 1e-5
NCORES = 8
ZP = S // NCORES     # 3 owned planes
PL = S * S           # 576
VC = ZP * PL         # 1728 owned voxels
Z7 = ZP + 4          # 7 input planes
Z5 = ZP + 2          # 5 planes for xb/xln
W5 = Z5 * PL         # 2880
PPL = 26 * 26        # 676 padded plane
HDR = 32             # headroom around padded slabs (keeps offsets even)
MVN = HDR + Z7 * PPL + HDR
HPN = HDR + Z5 * PPL + HDR
KFLAT = C * KK       # 20736
W2R = KFLAT // NCORES  # 2592
NB = 288
WSC = 16.0           # fp8 weight scale
GSC = 256.0          # fp8 gate scale

TAPS = [(dz, dy, dx) for dz in (-1, 0, 1) for dy in (-1, 0, 1) for dx in (-1, 0, 1)]


def _tidx(dz, dy, dx):
    return (dz + 1) * 9 + (dy + 1) * 3 + (dx + 1)


# DoubleRow tap pairs: the pair stride (byte delta between the two moving
# windows) must be EVEN, so pair dx=-1 with dx=+1 (delta 2), and the dx=0
# column across dz/dy (deltas 676 / 26).
PAIRS = ([(_tidx(dz, dy, -1), _tidx(dz, dy, 1))
          for dz in (-1, 0, 1) for dy in (-1, 0, 1)]
         + [(_tidx(-1, dy, 0), _tidx(0, dy, 0)) for dy in (-1, 0, 1)]
         + [(_tidx(1, -1, 0), _tidx(1, 0, 0)), (_tidx(1, 1, 0), None)])
NPR = len(PAIRS)     # 14

_CACHE = {}

Copy = mybir.ActivationFunctionType.Copy
Iden = mybir.ActivationFunctionType.Identity
Gelu = mybir.ActivationFunctionType.Gelu
Sigmoid = mybir.ActivationFunctionType.Sigmoid
Square = mybir.ActivationFunctionType.Square
Sqrt = mybir.ActivationFunctionType.Sqrt
Relu = mybir.ActivationFunctionType.Relu
ADD = mybir.AluOpType.add
SUB = mybir.AluOpType.subtract
MULT = mybir.AluOpType.mult
DR = mybir.MatmulPerfMode.DoubleRow

# smalls columns
(S_Y, S_BA, S_BB, S_BC, S_MODB, S_OPB, S_N2W, S_N2B, S_N3W, S_N3B,
 S_GNG, S_GNB, S_N2WL, S_N2BL, S_N2WH, S_N2BH) = range(16)


def _toff(dz, dy, dx):
    return dz * PPL + dy * 26 + dx


def build_program():
    nc = bacc.Bacc("TRN2", target_bir_lowering=False)

    def dram_in(name, shape, dtype=F32):
        return nc.declare_dram_parameter(name, list(shape), dtype, isOutput=False)

    x_halo = dram_in("x_halo", [C, Z7 * PL], BF16)
    smalls = dram_in("smalls", [C, 16])
    knb1 = dram_in("knb1", [HID])
    knb2r = dram_in("knb2r", [KFLAT])
    gind6 = dram_in("gind6", [CT, 128, G])
    gexpT = dram_in("gexpT", [G, C])
    ident = dram_in("ident", [128, 128], BF16)
    onesc = dram_in("onesc", [128, 1], BF16)
    wAT = dram_in("wAT", [C, C], BF16)
    wBT = dram_in("wBT", [C, C], BF16)
    wCT = dram_in("wCT", [C, C], BF16)
    modWT = dram_in("modWT", [2 * C, C], BF16)
    w1T = dram_in("w1T", [2 * C, HID], BF16)
    w2m = dram_in("w2m", [HT, 128, W2R], BF16)
    opT = dram_in("opT", [C, C], BF16)
    wi8 = dram_in("wi8", [HT, 128, 3 * 256], FP8)
    d8 = dram_in("d8", [HT, 128, NPR * 256], FP8)
    wo8 = dram_in("wo8", [CT, 128, 6 * 256], FP8)
    out = nc.declare_dram_parameter("out", [C, VC], F32, isOutput=True)
    import os as _os
    DBG = _os.environ.get("BLK3D_DBG") == "1"
    if DBG:
        dbg_mod = nc.declare_dram_parameter("dbg_mod", [C, 1], F32, isOutput=True)
        dbg_kern = nc.declare_dram_parameter("dbg_kern", [C, KK], F32, isOutput=True)
        dbg_dyn = nc.declare_dram_parameter("dbg_dyn", [C, W5], F32, isOutput=True)
        dbg_xb = nc.declare_dram_parameter("dbg_xb", [C, W5], F32, isOutput=True)
        dbg_xln = nc.declare_dram_parameter("dbg_xln", [3 * 128, 2 * W5], F32, isOutput=True)
        dbg_gate = nc.declare_dram_parameter("dbg_gate", [6 * 128, 2 * VC], F32, isOutput=True)
        dbg_y = nc.declare_dram_parameter("dbg_y", [C, VC], F32, isOutput=True)
        dbg_comb = nc.declare_dram_parameter("dbg_comb", [C, 2], F32, isOutput=True)
        dbg_kp1 = nc.declare_dram_parameter("dbg_kp1", [HID, 1], F32, isOutput=True)
        dbg_krow = nc.declare_dram_parameter("dbg_krow", [1, W2R], F32, isOutput=True)

    with tile.TileContext(nc) as tc, ExitStack() as ctx:
        dram = ctx.enter_context(tc.tile_pool(name="dram", bufs=1, space="DRAM"))
        persist = ctx.enter_context(tc.tile_pool(name="persist", bufs=1))
        gpool = ctx.enter_context(tc.tile_pool(name="gemv", bufs=2))

        # ---------------- persistent small tiles ----------------
        sm = [persist.tile([128, 16], F32, name=f"sm{i}", tag=f"sm{i}")
              for i in range(CT)]
        for i in range(CT):
            nc.sync.dma_start(sm[i][:], smalls[128 * i:128 * (i + 1), :])
        id_t = persist.tile([128, 128], BF16, name="identt", tag="identt")
        nc.sync.dma_start(id_t[:], ident[:, :])
        ones_t = persist.tile([128, 1], BF16, name="onest", tag="onest")
        nc.sync.dma_start(ones_t[:], onesc[:, :])
        eps_t = persist.tile([128, 1], F32, name="epst", tag="epst")
        nc.vector.memset(eps_t[:], EPS)
        junk = persist.tile([128, VC], BF16, name="junk", tag="junk")
        kern = [persist.tile([128, KK], F32, name=f"kern{i}", tag=f"kern{i}")
                for i in range(CT)]
        cb_cols = [persist.tile([128, 1], F32, name=f"cbc{m}", tag=f"cbc{m}")
                   for m in range(CT)]

        xbp = ctx.enter_context(tc.tile_pool(name="xbp", bufs=1))
        xb = [xbp.tile([128, W5], BF16, name=f"xb{i}", tag=f"xb{i}")
              for i in range(CT)]

        def ln_rows(pool, tiles, width, tag):
            """per-voxel mean/rstd over C -> bf16 bcast tiles [128, width]"""
            with (tc.tile_pool(name=f"{tag}ps", bufs=2, space="PSUM") as lps,
                  tc.tile_pool(name=f"{tag}sq", bufs=2) as sqp,
                  tc.tile_pool(name=f"{tag}rw", bufs=1) as rwp):
                nch = (width + 511) // 512
                row = rwp.tile([1, 2 * width], F32, name="row", tag="row")
                for cidx in range(nch):
                    o0 = 512 * cidx
                    n = min(512, width - o0)
                    ps1 = lps.tile([1, 512], F32, name="s1", tag="s1")
                    for k in range(CT):
                        nc.tensor.matmul(ps1[:, 0:n], ones_t[:],
                                         tiles[k][:, o0:o0 + n],
                                         start=(k == 0), stop=(k == CT - 1))
                    nc.scalar.activation(row[:, o0:o0 + n], ps1[:, 0:n], Copy,
                                         scale=1.0 / C)
                for cidx in range(nch):
                    o0 = 512 * cidx
                    n = min(512, width - o0)
                    ps2 = lps.tile([1, 512], F32, name="s2", tag="s2")
                    for k in range(CT):
                        sq = sqp.tile([128, 512], BF16, name="sq", tag="sq")
                        nc.vector.tensor_mul(sq[:, 0:n], tiles[k][:, o0:o0 + n],
                                             tiles[k][:, o0:o0 + n])
                        nc.tensor.matmul(ps2[:, 0:n], ones_t[:], sq[:, 0:n],
                                         start=(k == 0), stop=(k == CT - 1))
                    nc.scalar.activation(row[:, width + o0:width + o0 + n],
                                         ps2[:, 0:n], Copy, scale=1.0 / C)
                # spread each half to [96, w] for cheap elementwise math
                w96 = width // 96
                rs = rwp.tile([96, 2 * w96], F32, name="rs", tag="rs")
                nc.gpsimd.dma_start(rs[:, 0:w96], row[:, 0:width])
                nc.gpsimd.dma_start(rs[:, w96:2 * w96], row[:, width:2 * width])
                m2 = rwp.tile([96, w96], F32, name="m2", tag="m2")
                nc.scalar.square(m2[:], rs[:, 0:w96])
                vr = rwp.tile([96, w96], F32, name="vr", tag="vr")
                nc.vector.tensor_sub(vr[:], rs[:, w96:2 * w96], m2[:])
                nc.scalar.activation(vr[:], vr[:], Sqrt, bias=eps_t[0:96, 0:1])
                nc.vector.reciprocal(vr[:], vr[:])
                mrow = rwp.tile([1, 2 * width], BF16, name="mrow", tag="mrow")
                nc.gpsimd.dma_start(mrow[:, 0:width], rs[:, 0:w96])
                nc.gpsimd.dma_start(mrow[:, width:2 * width], vr[:])
                muB = pool.tile([128, width], BF16, name=f"{tag}mu",
                                tag=f"{tag}mu")
                rsB = pool.tile([128, width], BF16, name=f"{tag}rs",
                                tag=f"{tag}rs")
                nc.gpsimd.partition_broadcast(muB[:], mrow[0:1, 0:width])
                nc.gpsimd.partition_broadcast(rsB[:], mrow[0:1, width:2 * width])
            return muB, rsB

        # =================== phases A-C: scoped (xs/mv/dyn die after) ======
        with (tc.tile_pool(name="xsp", bufs=1) as xsp,
              tc.tile_pool(name="dynp", bufs=1) as dynp,
              tc.tile_pool(name="opTp", bufs=1) as opTp):
            xs = [xsp.tile([128, Z7 * PL], BF16, name=f"xs{i}", tag=f"xs{i}")
                  for i in range(CT)]
            for i in range(CT):
                nc.sync.dma_start(xs[i][:], x_halo[128 * i:128 * (i + 1), :])
            dyn = [dynp.tile([128, W5], BF16, name=f"dyn{i}", tag=f"dyn{i}")
                   for i in range(CT)]
            opT_t = [opTp.tile([128, C], BF16, name=f"opT{i}", tag=f"opT{i}")
                     for i in range(CT)]
            for i in range(CT):
                nc.sync.dma_start(opT_t[i][:], opT[128 * i:128 * (i + 1), :])

            # ---- vc partials + AG1 ----
            vcs = persist.tile([128, CT], F32, name="vcs", tag="vcs")
            for i in range(CT):
                nc.scalar.activation(junk[:], xs[i][:, 2 * PL:2 * PL + VC],
                                     Copy, accum_out=vcs[:, i:i + 1])
            ag1i = dram.tile([C], F32, name="ag1i", tag="ag1i")
            ag1o = dram.tile([NCORES, C], F32, name="ag1o", tag="ag1o",
                             addr_space="Shared")
            nc.gpsimd.dma_start(
                bass.AP(tensor=ag1i[:].tensor, offset=ag1i[:].offset,
                        ap=[[1, 128], [128, CT]]), vcs[:])
            nc.gpsimd.collective_compute(
                "AllGather", mybir.AluOpType.bypass,
                replica_groups=[list(range(NCORES))], ins=[ag1i[:]],
                outs=[ag1o[:]])

            # ---- text chain / mod / kp1 (row-form gemv) ----
            _psRs = ExitStack()
            psR = _psRs.enter_context(
                tc.tile_pool(name="psR", bufs=1, space="PSUM"))
            psT = _psRs.enter_context(
                tc.tile_pool(name="psT", bufs=2, space="PSUM"))
            pcs = [psR.tile([1, 512], F32, name=f"rp{j}", tag=f"rowps{j}")
                   for j in range(6)]
            with tc.tile_pool(name="wstream", bufs=2) as wpool:

                def gemv_chain(wdram, in_cols, nk, nm, act, bias_col, tag,
                               odt=BF16, bias_t=None):
                    # out = W @ in  via moving-weights row matmuls; the row
                    # [1, 128*nm] is transposed to columns through DRAM.
                    width = 128 * nm
                    npc = (width + 511) // 512
                    for k in range(nk):
                        wt = wpool.tile([128, width], BF16, name=f"{tag}w",
                                        tag=f"{tag}w")
                        nc.sync.dma_start(wt[:],
                                          wdram[128 * k:128 * (k + 1), :])
                        for pc in range(npc):
                            o0 = 512 * pc
                            n = min(512, width - o0)
                            nc.tensor.matmul(pcs[pc][:, 0:n], in_cols[k][:],
                                             wt[:, o0:o0 + n],
                                             start=(k == 0),
                                             stop=(k == nk - 1))
                    row = gpool.tile([1, width], BF16, name=f"{tag}row",
                                     tag=f"{tag}row")
                    for pc in range(npc):
                        o0 = 512 * pc
                        n = min(512, width - o0)
                        nc.scalar.activation(row[:, o0:o0 + n],
                                             pcs[pc][:, 0:n], Copy)
                    # row -> columns via K=1 matmuls (stationary = row slice,
                    # moving = a single 1.0): out[p, 0] = row[0, 128m + p]
                    outs = []
                    for m in range(nm):
                        cps = psT.tile([128, 1], F32, name=f"{tag}tc",
                                       tag="tcol")
                        nc.tensor.matmul(cps[:], row[:, 128 * m:128 * (m + 1)],
                                         ones_t[0:1, 0:1], start=True,
                                         stop=True)
                        if bias_t is not None:
                            bias = bias_t[:, m:m + 1]
                        else:
                            bias = sm[m][:, bias_col:bias_col + 1]
                        o = gpool.tile([128, 1], odt, name=f"{tag}o{m}",
                                       tag=f"{tag}o{m}")
                        nc.scalar.activation(o[:], cps[:], act, bias=bias)
                        outs.append(o)
                    return outs

                y_cols = []
                for i in range(CT):
                    t = gpool.tile([128, 1], BF16, name=f"yc{i}", tag=f"yc{i}")
                    nc.scalar.activation(t[:], sm[i][:, S_Y:S_Y + 1], Copy)
                    y_cols.append(t)
                hA = gemv_chain(wAT, y_cols, CT, CT, Relu, S_BA, "wa")
                hB = gemv_chain(wBT, hA, CT, CT, Iden, S_BB, "wb")
                attn = gemv_chain(wCT, hB, CT, CT, Iden, S_BC, "wc")

                vc8 = persist.tile([128, CT * NCORES], F32, name="vc8",
                                   tag="vc8")
                nc.gpsimd.dma_start(
                    bass.AP(tensor=vc8[:].tensor, offset=vc8[:].offset,
                            ap=[vc8[:].ap[0], [CT, NCORES], [1, CT]]),
                    bass.AP(tensor=ag1o[:].tensor, offset=ag1o[:].offset,
                            ap=[[1, 128], [C, NCORES], [128, CT]]))
                v3 = vc8.rearrange("p (i k) -> p i k", i=NCORES, k=CT)
                for step in (4, 2, 1):
                    nc.vector.tensor_add(v3[:, 0:step, :], v3[:, 0:step, :],
                                         v3[:, step:2 * step, :])
                comb = []
                for i in range(CT):
                    cb = gpool.tile([128, 1], BF16, name=f"cmb{i}",
                                    tag=f"cmb{i}")
                    nc.scalar.activation(cb[:], vc8[:, i:i + 1], Copy,
                                         scale=1.0 / V)
                    comb.append(cb)
                comb += attn

                mod = gemv_chain(modWT, comb, 2 * CT, CT, Sigmoid, S_MODB,
                                 "md", odt=F32)

                knb1_t = persist.tile([128, HT], F32, name="knb1t",
                                      tag="knb1t")
                nc.gpsimd.dma_start(
                    knb1_t[:],
                    bass.AP(tensor=knb1, offset=0, ap=[[1, 128], [128, HT]]))
                kp1 = gemv_chain(w1T, comb, 2 * CT, HT, Relu, None, "k1",
                                 bias_t=knb1_t)

            if DBG:
                with tc.tile_pool(name="dbg0p", bufs=2) as dbg0p:
                    for i in range(CT):
                        t = dbg0p.tile([128, 2], F32, name="dbgc", tag="dbgc")
                        nc.scalar.activation(t[:, 0:1], comb[i][:], Copy)
                        nc.scalar.activation(t[:, 1:2], comb[CT + i][:], Copy)
                        nc.gpsimd.dma_start(dbg_comb[128 * i:128 * (i + 1), :],
                                            t[:])
                    for m in range(HT):
                        t = dbg0p.tile([128, 1], F32, name="dbgk1", tag="dbgk1")
                        nc.scalar.activation(t[:], kp1[m][:], Copy)
                        nc.gpsimd.dma_start(
                            dbg_kp1[128 * m:128 * (m + 1), :], t[:])

            # ---- W2 own rows + AG2 -> kernels ----
            ag2i = dram.tile([W2R], F32, name="ag2i", tag="ag2i")
            ag2o = dram.tile([NCORES, W2R], F32, name="ag2o", tag="ag2o",
                             addr_space="Shared")
            PIECES = [(0, 512), (512, 512), (1024, 512), (1536, 512),
                      (2048, 512), (2560, 32)]
            with (tc.tile_pool(name="w2s", bufs=3) as w2s,
                  tc.tile_pool(name="krowp", bufs=1) as krowp):
                pcs = [psR.tile([1, 512], F32, name=f"w2p{j}",
                                tag=f"rowps{j}") for j in range(6)]
                for k in range(HT):
                    wt = w2s.tile([128, W2R], BF16, name="w2w", tag="w2w")
                    nc.sync.dma_start(wt[:], w2m[k, :, :])
                    for j, (o0, n) in enumerate(PIECES):
                        nc.tensor.matmul(pcs[j][:, 0:n], kp1[k][:],
                                         wt[:, o0:o0 + n], start=(k == 0),
                                         stop=(k == HT - 1))
                krow = krowp.tile([1, W2R], F32, name="krow", tag="krow")
                for j, (o0, n) in enumerate(PIECES):
                    nc.scalar.activation(krow[:, o0:o0 + n], pcs[j][:, 0:n],
                                         Copy)
                nc.gpsimd.dma_start(ag2i[:], krow[:])
                if DBG:
                    nc.gpsimd.dma_start(dbg_krow[:, :], krow[:])
            nc.gpsimd.collective_compute(
                "AllGather", mybir.AluOpType.bypass,
                replica_groups=[list(range(NCORES))], ins=[ag2i[:]],
                outs=[ag2o[:]])
            for i in range(CT):
                nc.gpsimd.dma_start(
                    kern[i][:],
                    bass.AP(tensor=ag2o[:].tensor,
                            offset=ag2o[:].offset + 128 * i * KK,
                            ap=[[KK, 128], [1, KK]]))
                kb = gpool.tile([128, KK], F32, name="kernb", tag="kernb")
                nc.gpsimd.dma_start(
                    kb[:], bass.AP(tensor=knb2r, offset=128 * i * KK,
                                   ap=[[KK, 128], [1, KK]]))
                nc.vector.tensor_add(kern[i][:], kern[i][:], kb[:])
            _psRs.close()

            # ---- mv staging + dyn conv + GN stats ----
            ag3i = dram.tile([G, 2], F32, name="ag3i", tag="ag3i")
            ag3o = dram.tile([NCORES, G, 2], F32, name="ag3o", tag="ag3o",
                             addr_space="Shared")
            with (tc.tile_pool(name="mvp", bufs=3) as mvp,
                  tc.tile_pool(name="diagp", bufs=2) as diagp,
                  tc.tile_pool(name="cvps", bufs=6, space="PSUM") as cvps,
                  tc.tile_pool(name="gnps", bufs=1, space="PSUM") as gnps,
                  tc.tile_pool(name="gnst", bufs=2) as gnst):
                gps = gnps.tile([G, 2], F32, name="gps", tag="gps")
                for i in range(CT):
                    mv = mvp.tile([128, MVN], BF16, name="mv", tag="mv")
                    if i < 3:
                        nc.gpsimd.memset(mv[:], 0.0)
                    for z in range(Z7):
                        for yh in range(2):
                            dst = bass.AP(
                                tensor=mv[:].tensor,
                                offset=(mv[:].offset + HDR + z * PPL
                                        + yh * 312),
                                ap=[mv[:].ap[0], [26, 12], [1, 24]])
                            nc.vector.tensor_scalar_mul(
                                dst,
                                xs[i][:, z * PL + yh * NB:
                                      z * PL + yh * NB + NB],
                                mod[i][:, 0:1])
                    dg = [diagp.tile([128, 128], BF16, name=f"dg{t}",
                                     tag=f"dg{t}") for t in range(KK)]
                    for t in range(KK):
                        nc.vector.tensor_scalar_mul(dg[t][:], id_t[:],
                                                    kern[i][:, t:t + 1])
                    for o in range(Z5):
                        for bh in range(2):
                            ps = cvps.tile([128, NB], F32, name="cv",
                                           tag="cv")
                            base = (mv[:].offset + HDR + o * PPL + bh * 312)
                            for t, (dz, dy, dx) in enumerate(TAPS):
                                sv = bass.AP(
                                    tensor=mv[:].tensor,
                                    offset=base + _toff(1 + dz, dy, dx),
                                    ap=[mv[:].ap[0], [26, 12], [1, 24]])
                                nc.tensor.matmul(ps[:], dg[t][:], sv,
                                                 start=(t == 0),
                                                 stop=(t == KK - 1))
                            nc.scalar.activation(
                                dyn[i][:, o * PL + bh * NB:
                                       o * PL + bh * NB + NB],
                                ps[:], Copy)
                    st = gnst.tile([128, 2], F32, name="gnstat", tag="gnstat")
                    nc.scalar.activation(junk[:], dyn[i][:, PL:PL + VC], Copy,
                                         accum_out=st[:, 0:1])
                    nc.scalar.activation(junk[:], dyn[i][:, PL:PL + VC],
                                         Square, accum_out=st[:, 1:2])
                    gi = gnst.tile([128, G], F32, name="gind", tag="gind")
                    nc.gpsimd.dma_start(gi[:], gind6[i, :, :])
                    nc.tensor.matmul(gps[:], gi[:], st[:], start=(i == 0),
                                     stop=(i == CT - 1))
                gsb = persist.tile([G, 2], F32, name="gsb", tag="gsb")
                nc.scalar.activation(gsb[:], gps[:], Copy)
            nc.gpsimd.dma_start(ag3i[:], gsb[:])
            nc.gpsimd.collective_compute(
                "AllGather", mybir.AluOpType.bypass,
                replica_groups=[list(range(NCORES))], ins=[ag3i[:]],
                outs=[ag3o[:]])

            gst8 = persist.tile([G, 2 * NCORES], F32, name="gst8", tag="gst8")
            nc.gpsimd.dma_start(
                bass.AP(tensor=gst8[:].tensor, offset=gst8[:].offset,
                        ap=[gst8[:].ap[0], [2, NCORES], [1, 2]]),
                bass.AP(tensor=ag3o[:].tensor, offset=ag3o[:].offset,
                        ap=[[2, G], [2 * G, NCORES], [1, 2]]))
            g3 = gst8.rearrange("p (i t) -> p i t", i=NCORES, t=2)
            for step in (4, 2, 1):
                nc.vector.tensor_add(g3[:, 0:step, :], g3[:, 0:step, :],
                                     g3[:, step:2 * step, :])
            NGRP = float(GD * V)
            gmr = persist.tile([G, 2], F32, name="gmr", tag="gmr")
            nc.scalar.activation(gmr[:, 0:1], gst8[:, 0:1], Copy,
                                 scale=1.0 / NGRP)
            musq = persist.tile([G, 1], F32, name="musq", tag="musq")
            nc.scalar.square(musq[:], gmr[:, 0:1])
            var = persist.tile([G, 1], F32, name="gvar", tag="gvar")
            nc.vector.tensor_scalar(var[:], gst8[:, 1:2], 1.0 / NGRP, None,
                                    op0=MULT)
            nc.vector.tensor_sub(var[:], var[:], musq[:])
            nc.scalar.activation(var[:], var[:], Sqrt, bias=eps_t[0:G, 0:1])
            nc.vector.reciprocal(gmr[:, 1:2], var[:])

            with (tc.tile_pool(name="gnf", bufs=2) as gnf,
                  tc.tile_pool(name="psA", bufs=2, space="PSUM") as psA):
                shifts = []
                gscs = []
                for i in range(CT):
                    ge = gnf.tile([G, 128], F32, name=f"gexp{i}",
                                  tag=f"gexp{i}")
                    nc.gpsimd.dma_start(ge[:], gexpT[:, 128 * i:128 * (i + 1)])
                    ps = psA.tile([128, 2], F32, name="gn2", tag="gvps")
                    nc.tensor.matmul(ps[:], ge[:], gmr[:], start=True,
                                     stop=True)
                    mu_c = gnf.tile([128, 2], F32, name=f"muc{i}",
                                    tag=f"muc{i}")
                    nc.scalar.activation(mu_c[:], ps[:], Copy)
                    a = persist.tile([128, 1], F32, name=f"gsc{i}",
                                     tag=f"gsc{i}")
                    nc.vector.tensor_mul(a[:], sm[i][:, S_GNG:S_GNG + 1],
                                         mu_c[:, 1:2])
                    b = gnf.tile([128, 1], BF16, name=f"gsh{i}", tag=f"gsh{i}")
                    t = gnf.tile([128, 1], F32, name="gtmp", tag="gtmp")
                    nc.vector.tensor_mul(t[:], mu_c[:, 0:1], a[:])
                    nc.vector.tensor_sub(t[:], sm[i][:, S_GNB:S_GNB + 1], t[:])
                    nc.scalar.activation(b[:], t[:], Copy)
                    shifts.append(b)
                    gscs.append(a)
                for m in range(CT):
                    ps = psA.tile([128, 1], F32, name="cbp", tag="gvps")
                    for k in range(CT):
                        nc.tensor.matmul(ps[:],
                                         opT_t[k][:, 128 * m:128 * (m + 1)],
                                         shifts[k][:], start=(k == 0),
                                         stop=(k == CT - 1))
                    nc.scalar.activation(cb_cols[m][:], ps[:], Iden,
                                         bias=sm[m][:, S_OPB:S_OPB + 1])
                for i in range(CT):
                    nc.vector.tensor_scalar_mul(opT_t[i][:], opT_t[i][:],
                                                gscs[i][:])

            # cafm matmul (5 planes) + xb = (psum + cb) * x
            CH5 = [(0, 512), (512, 512), (1024, 512), (1536, 512),
                   (2048, 512), (2560, 320)]
            with tc.tile_pool(name="opwps", bufs=2, space="PSUM") as opwps:
                for m in range(CT):
                    for o0, n in CH5:
                        ps = opwps.tile([128, 512], F32, name="opw", tag="opw")
                        for k in range(CT):
                            nc.tensor.matmul(
                                ps[:, 0:n],
                                opT_t[k][:, 128 * m:128 * (m + 1)],
                                dyn[k][:, o0:o0 + n], start=(k == 0),
                                stop=(k == CT - 1))
                        nc.vector.scalar_tensor_tensor(
                            xb[m][:, o0:o0 + n], ps[:, 0:n], cb_cols[m][:],
                            xs[m][:, PL + o0:PL + o0 + n], op0=ADD, op1=MULT)

            if DBG:
                with tc.tile_pool(name="dbgp", bufs=2) as dbgp:
                    for i in range(CT):
                        t = dbgp.tile([128, W5], F32, name="dbgt", tag="dbgt")
                        nc.scalar.activation(t[:], dyn[i][:], Copy)
                        nc.gpsimd.dma_start(dbg_dyn[128 * i:128 * (i + 1), :],
                                            t[:])
                        t2 = dbgp.tile([128, W5], F32, name="dbgt2",
                                       tag="dbgt2")
                        nc.scalar.activation(t2[:], xb[i][:], Copy)
                        nc.gpsimd.dma_start(dbg_xb[128 * i:128 * (i + 1), :],
                                            t2[:])
                        nc.gpsimd.dma_start(dbg_kern[128 * i:128 * (i + 1), :],
                                            kern[i][:])
                        t3 = dbgp.tile([128, 1], F32, name="dbgt3",
                                       tag="dbgt3")
                        nc.scalar.activation(t3[:], mod[i][:], Copy)
                        nc.gpsimd.dma_start(dbg_mod[128 * i:128 * (i + 1), :],
                                            t3[:])

        # =================== LN2 -> xln8 (fp8, channel-paired) =============
        xln8p = ctx.enter_context(tc.tile_pool(name="xln8p", bufs=1))
        xln8 = [xln8p.tile([128, 2 * W5], FP8, name=f"xl{q}", tag=f"xl{q}")
                for q in range(3)]
        with tc.tile_pool(name="lnbp", bufs=1) as lnbp:
            muB, rsB = ln_rows(lnbp, xb, W5, "ln2")
            with tc.tile_pool(name="lnt", bufs=2) as lnt:
                for i in range(CT):
                    t1 = lnt.tile([128, W5], BF16, name="lnt1", tag="lnt1")
                    nc.vector.tensor_sub(t1[:], xb[i][:], muB[:])
                    nc.vector.tensor_mul(t1[:], t1[:], rsB[:])
                    q, j = i // 2, i % 2
                    dst = xln8[q][:, j * W5:(j + 1) * W5]
                    nc.scalar.activation(dst[:, 0:PL], t1[:, 0:PL], Iden,
                                         bias=sm[i][:, S_N2BL:S_N2BL + 1],
                                         scale=sm[i][:, S_N2WL:S_N2WL + 1])
                    nc.scalar.activation(dst[:, PL:4 * PL], t1[:, PL:4 * PL],
                                         Iden,
                                         bias=sm[i][:, S_N2B:S_N2B + 1],
                                         scale=sm[i][:, S_N2W:S_N2W + 1])
                    nc.scalar.activation(dst[:, 4 * PL:5 * PL],
                                         t1[:, 4 * PL:5 * PL], Iden,
                                         bias=sm[i][:, S_N2BH:S_N2BH + 1],
                                         scale=sm[i][:, S_N2WH:S_N2WH + 1])

        # =================== MLP: fp8 DoubleRow ===========================
        gate8p = ctx.enter_context(tc.tile_pool(name="gate8p", bufs=1))
        gate8 = [gate8p.tile([128, 2 * VC], FP8, name=f"g8{q}", tag=f"g8{q}")
                 for q in range(6)]
        ytp = ctx.enter_context(tc.tile_pool(name="ytp", bufs=1))
        y_t = [ytp.tile([128, VC], BF16, name=f"y{i}", tag=f"y{i}")
               for i in range(CT)]

        pair_off = []
        for t0, t1 in PAIRS:
            o0 = _toff(1 + TAPS[t0][0], TAPS[t0][1], TAPS[t0][2])
            if t1 is not None:
                d = _toff(1 + TAPS[t1][0], TAPS[t1][1], TAPS[t1][2]) - o0
            else:
                d = 2  # dead pair slot (zero weights), even stride
            pair_off.append((o0, d))
        assert all(d > 0 and d % 2 == 0 for _, d in pair_off)

        with (tc.tile_pool(name="hpadp", bufs=4) as hpad_pool,
              tc.tile_pool(name="wi8p", bufs=4) as wi8p,
              tc.tile_pool(name="d8p", bufs=4) as d8p,
              tc.tile_pool(name="glueG", bufs=3) as glueG,
              tc.tile_pool(name="wips", bufs=3, space="PSUM") as wips,
              tc.tile_pool(name="cvp2", bufs=5, space="PSUM") as cvp2):

            nmlp = [0]

            def mlp_tile(tt, sink):
                """Wi (fp8 DR) -> staged padded h8 -> conv (fp8 DR pairs);
                sink(nb, ps) consumes each conv psum block immediately."""
                wt = wi8p.tile([128, 3 * 256], FP8, name="wi8t", tag="wi8t")
                nc.sync.dma_start(wt[:], wi8[tt, :, :])
                w4 = wt.rearrange("p (q j m) -> p q j m", q=3, j=2, m=128)
                hp = hpad_pool.tile([128, HPN], FP8, name="hpad", tag="hpad")
                if nmlp[0] < 4:
                    nc.gpsimd.memset(hp[:], 0.0)
                nmlp[0] += 1
                for z in range(Z5):
                    for yh in range(2):
                        ps = wips.tile([128, NB], F32, name="wi_ps",
                                       tag="wi_ps")
                        for q in range(3):
                            mv_ = bass.AP(
                                tensor=xln8[q][:].tensor,
                                offset=(xln8[q][:].offset + z * PL + yh * NB),
                                ap=[xln8[q][:].ap[0], [W5, 2], [1, NB]])
                            nc.tensor.matmul(ps[:], w4[:, q], mv_,
                                             start=(q == 0), stop=(q == 2),
                                             perf_mode=DR)
                        dst = bass.AP(
                            tensor=hp[:].tensor,
                            offset=hp[:].offset + HDR + z * PPL + yh * 312,
                            ap=[hp[:].ap[0], [26, 12], [1, 24]])
                        if z in (1, 3):
                            nc.vector.tensor_scalar_mul(dst, ps[:], 1.0 / WSC)
                        else:
                            nc.scalar.activation(dst, ps[:], Copy,
                                                 scale=1.0 / WSC)
                dgt = d8p.tile([128, NPR * 256], FP8, name="d8t", tag="d8t")
                nc.sync.dma_start(dgt[:], d8[tt, :, :])
                dg4 = dgt.rearrange("p (r j m) -> p r j m", r=NPR, j=2, m=128)
                for o in range(ZP):
                    for bh in range(2):
                        # stream full padded rows (312 wide) so the moving AP
                        # stays 3-dim; consumer reads interior cells strided
                        ps = cvp2.tile([128, 312], F32, name="cv2", tag="cv2")
                        base = hp[:].offset + HDR + (o + 1) * PPL + bh * 312
                        for pr in range(NPR):
                            o0, dlt = pair_off[pr]
                            sv = bass.AP(
                                tensor=hp[:].tensor, offset=base + o0 - PPL,
                                ap=[hp[:].ap[0], [dlt, 2], [1, 312]])
                            nc.tensor.matmul(ps[:], dg4[:, pr], sv,
                                             start=(pr == 0),
                                             stop=(pr == NPR - 1),
                                             perf_mode=DR)
                        sink(o * 2 + bh, ps)

            for u in range(HH):
                def _interior(ps):
                    return bass.AP(tensor=ps[:].tensor, offset=ps[:].offset,
                                   ap=[ps[:].ap[0], [26, 12], [1, 24]])

                g1 = glueG.tile([128, VC], BF16, name="gelu1", tag="gelu1")
                mlp_tile(u, lambda nb, ps: nc.scalar.activation(
                    g1[:, NB * nb:NB * (nb + 1)], _interior(ps), Gelu,
                    scale=1.0 / WSC))
                q, j = u // 2, u % 2
                # gate = (gelu(c1) * GSC/WSC) * conv2, reading conv2 psum
                # blocks directly (scale folded: GSC/WSC applied to g1)
                mlp_tile(u + HH, lambda nb, ps: nc.vector.scalar_tensor_tensor(
                    gate8[q][:, j * VC + NB * nb:j * VC + NB * (nb + 1)],
                    g1[:, NB * nb:NB * (nb + 1)], GSC / WSC, _interior(ps),
                    op0=MULT, op1=MULT))

        with (tc.tile_pool(name="wo8p", bufs=2) as wo8p,
              tc.tile_pool(name="wops", bufs=2, space="PSUM") as wops):
            CH3 = [(0, 512), (512, 512), (1024, 512), (1536, 192)]
            for m in range(CT):
                wt = wo8p.tile([128, 6 * 256], FP8, name="wo8t", tag="wo8t")
                nc.sync.dma_start(wt[:], wo8[m, :, :])
                w4 = wt.rearrange("p (q j m) -> p q j m", q=6, j=2, m=128)
                for o0, n in CH3:
                    ps = wops.tile([128, 512], F32, name="wo_ps", tag="wo_ps")
                    for q in range(6):
                        mv_ = bass.AP(
                            tensor=gate8[q][:].tensor,
                            offset=gate8[q][:].offset + o0,
                            ap=[gate8[q][:].ap[0], [VC, 2], [1, n]])
                        nc.tensor.matmul(ps[:, 0:n], w4[:, q], mv_,
                                         start=(q == 0), stop=(q == 5),
                                         perf_mode=DR)
                    nc.vector.scalar_tensor_tensor(
                        y_t[m][:, o0:o0 + n], ps[:, 0:n],
                        1.0 / (WSC * GSC), xb[m][:, PL + o0:PL + o0 + n],
                        op0=MULT, op1=ADD)

        if DBG:
            with tc.tile_pool(name="dbg2p", bufs=1) as dbg2p:
                for q in range(3):
                    t = dbg2p.tile([128, 2 * W5], F32, name="dbgx", tag="dbgx")
                    nc.scalar.activation(t[:], xln8[q][:], Copy)
                    nc.gpsimd.dma_start(dbg_xln[128 * q:128 * (q + 1), :], t[:])
                for q in range(6):
                    t = dbg2p.tile([128, 2 * VC], F32, name="dbgg", tag="dbgg")
                    nc.scalar.activation(t[:], gate8[q][:], Copy)
                    nc.gpsimd.dma_start(dbg_gate[128 * q:128 * (q + 1), :], t[:])
                for i in range(CT):
                    t = dbg2p.tile([128, VC], F32, name="dbgy", tag="dbgy")
                    nc.scalar.activation(t[:], y_t[i][:], Copy)
                    nc.gpsimd.dma_start(dbg_y[128 * i:128 * (i + 1), :], t[:])

        # =================== LN3 + output ==================================
        with tc.tile_pool(name="ln3bp", bufs=1) as ln3bp:
            muB3, rsB3 = ln_rows(ln3bp, y_t, VC, "ln3")
            with tc.tile_pool(name="glueH", bufs=2) as glueH:
                for i in range(CT):
                    t1 = glueH.tile([128, VC], BF16, name="ln3t", tag="ln3t")
                    nc.vector.tensor_sub(t1[:], y_t[i][:], muB3[:])
                    nc.vector.tensor_mul(t1[:], t1[:], rsB3[:])
                    of = glueH.tile([128, VC], F32, name="outf", tag="outf")
                    nc.scalar.activation(of[:], t1[:], Iden,
                                         bias=sm[i][:, S_N3B:S_N3B + 1],
                                         scale=sm[i][:, S_N3W:S_N3W + 1])
                    nc.gpsimd.dma_start(out[128 * i:128 * (i + 1), :], of[:])

    nc.compile()
    return nc


def _prep(inputs):
    bf = lambda a: np.ascontiguousarray(a).astype(BF)
    f32 = lambda a: np.ascontiguousarray(a, dtype=np.float32)
    fp8 = lambda a: np.ascontiguousarray(a).astype(E4)
    x = f32(inputs["x"][0])
    xf = x.reshape(C, S, PL)

    W_B = f32(inputs["tp_W"]) @ f32(inputs["lora_W2"])
    b_B = f32(inputs["tp_W"]) @ f32(inputs["lora_b2"]) + f32(inputs["tp_b"])
    W_C = f32(inputs["attn_Wo"]) @ f32(inputs["attn_Wv"])
    b_C = f32(inputs["attn_Wo"]) @ f32(inputs["attn_bv"]) + f32(inputs["attn_bo"])

    gind6 = np.zeros((CT, 128, G), np.float32)
    for j in range(CT):
        for p in range(128):
            gind6[j, p, (128 * j + p) // GD] = 1.0
    gexpT = np.zeros((G, C), np.float32)
    for c in range(C):
        gexpT[c // GD, c] = 1.0

    kn_W2 = f32(inputs["kn_W2"])
    kn_W1 = f32(inputs["kn_W1"])
    mlp_Wi = f32(inputs["mlp_Wi"])
    mlp_dw = f32(inputs["mlp_dw"]).reshape(HID, KK)
    mlp_Wo = f32(inputs["mlp_Wo"])

    wi8a = np.zeros((HT, 128, 3 * 256), np.float32)
    WiT = mlp_Wi.T
    for h in range(HT):
        blk = WiT[:, 128 * h:128 * (h + 1)]
        for q in range(3):
            for j in range(2):
                wi8a[h, :, q * 256 + j * 128:q * 256 + j * 128 + 128] = \
                    WSC * blk[256 * q + 128 * j:256 * q + 128 * (j + 1), :]
    d8a = np.zeros((HT, 128, NPR * 256), np.float32)
    idx = np.arange(128)
    for h in range(HT):
        for pr, (t0, t1) in enumerate(PAIRS):
            for j, t in enumerate((t0, t1)):
                if t is None:
                    continue
                d8a[h, idx, pr * 256 + j * 128 + idx] = \
                    WSC * mlp_dw[128 * h:128 * (h + 1), t]
    wo8a = np.zeros((CT, 128, 6 * 256), np.float32)
    WoT = mlp_Wo.T
    for m in range(CT):
        blk = WoT[:, 128 * m:128 * (m + 1)]
        for q in range(6):
            for j in range(2):
                wo8a[m, :, q * 256 + j * 128:q * 256 + j * 128 + 128] = \
                    WSC * blk[256 * q + 128 * j:256 * q + 128 * (j + 1), :]

    com = dict(
        gind6=gind6, gexpT=gexpT,
        ident=bf(np.eye(128, dtype=np.float32)),
        onesc=bf(np.ones((128, 1), np.float32)),
        wAT=bf(f32(inputs["lora_W1"]).T),
        wBT=bf(W_B.T), wCT=bf(W_C.T),
        modWT=bf(f32(inputs["mod_W"]).T),
        w1T=bf(kn_W1.T),
        knb1=f32(inputs["kn_b1"]), knb2r=f32(inputs["kn_b2"]),
        opT=bf(f32(inputs["op_W"]).T),
        wi8=fp8(wi8a), d8=fp8(d8a), wo8=fp8(wo8a),
    )

    n2w = f32(inputs["n2_w"]); n2b = f32(inputs["n2_b"])
    in_maps = []
    for i in range(NCORES):
        z0 = ZP * i
        xh = np.zeros((C, Z7, PL), np.float32)
        lo, hi = max(z0 - 2, 0), min(z0 + ZP + 2, S)
        xh[:, lo - (z0 - 2):lo - (z0 - 2) + (hi - lo)] = xf[:, lo:hi]
        lomask = 1.0 if i > 0 else 0.0
        himask = 1.0 if i < NCORES - 1 else 0.0
        smalls = np.zeros((C, 16), np.float32)
        smalls[:, S_Y] = f32(inputs["y"][0, 0])
        smalls[:, S_BA] = f32(inputs["lora_b1"])
        smalls[:, S_BB] = b_B
        smalls[:, S_BC] = b_C
        smalls[:, S_MODB] = f32(inputs["mod_b"])
        smalls[:, S_OPB] = f32(inputs["op_b"])
        smalls[:, S_N2W] = n2w
        smalls[:, S_N2B] = n2b
        smalls[:, S_N3W] = f32(inputs["n3_w"])
        smalls[:, S_N3B] = f32(inputs["n3_b"])
        smalls[:, S_GNG] = f32(inputs["gn_g"])
        smalls[:, S_GNB] = f32(inputs["gn_b"])
        smalls[:, S_N2WL] = n2w * lomask
        smalls[:, S_N2BL] = n2b * lomask
        smalls[:, S_N2WH] = n2w * himask
        smalls[:, S_N2BH] = n2b * himask
        m = dict(com)
        m.update(
            x_halo=xh.reshape(C, Z7 * PL).astype(BF),
            smalls=smalls,
            w2m=bf(kn_W2[W2R * i:W2R * (i + 1), :].T.reshape(HT, 128, W2R)),
        )
        in_maps.append(m)
    return in_maps


def kernel(**inputs) -> np.ndarray:
    if "nc" not in _CACHE:
        _CACHE["nc"] = build_program()
    nc = _CACHE["nc"]
    in_maps = _prep(inputs)
    res = run_bass_kernel_spmd(nc, in_maps, list(range(NCORES)))
    outs = [res.results[i]["out"].reshape(C, ZP, PL) for i in range(NCORES)]
    full = np.concatenate(outs, axis=1)
    return full.reshape(1, C, S, S, S).astype(np.float32)
